# revision 1
# baseline (speedup 1.0000x reference)
"""CondLaneHead DynamicMaskHead kernel for 8 Trainium2 NeuronCores.

Problem: per-instance 3-layer 1x1-conv MLP over a [64,160,256] feature map.
  feats = concat([loc_x, loc_y], x[img])            # [66, L], L = 160*256
  h1 = relu(w0 @ feats + b0)                        # [64, L]
  h2 = relu(w1 @ h1 + b1)                           # [64, L]
  out = w2 @ h2 + b2 - 2.19                         # [1, L]
32 instances (8 per image, 4 images). Sharding: 4 instances per core; each
core needs exactly one image's feature map.

Device mapping (per core):
  - The 4 instances form 2 pairs. Layer 1: one matmul per pair with
    lhsT = [67, 128] (2 coord rows + ones row for the bias + 64 x rows).
    Layer 2: block-diagonal [128, 128] weights, one matmul per pair.
  - Layer 3 (64->1): output is packed across PSUM partitions. Matmuls write
    [32, 512] windows at partition bases 0/32/64/96 using zero-padded weight
    variants (w2 placed at columns 2j of window j), accumulating over 16
    position-groups per window, so one PSUM bank collects 64 groups x 2
    instances = a full [128, 512] tile before a single bias+copy op and one
    scatter-DMA to DRAM.
  - dtypes: layer 1 runs in float32r (full-rate fp32 storage on the PE),
    h1/h2 and layer-2/3 weights in bf16, all PSUM accumulation in fp32.
"""

import sys

if "/opt/trn_rl_repo" not in sys.path:
    sys.path.insert(0, "/opt/trn_rl_repo")

import numpy as np
import ml_dtypes

import concourse.bass as bass
import concourse.mybir as mybir
from concourse import bacc
from concourse.tile import TileContext
from concourse.bass_utils import run_bass_kernel_spmd

BF = mybir.dt.bfloat16
F32 = mybir.dt.float32
F32R = mybir.dt.float32r
AT = mybir.ActivationFunctionType
AL = mybir.AluOpType
bf16 = ml_dtypes.bfloat16

# Problem geometry (hardcoded per spec)
N_IMG, INS_PER_IMG, C, H, W = 4, 8, 64, 160, 256
CIN = C + 2
L = H * W                       # 40960 positions
L1, L2_, L3_ = (C + 2) * C, C * C, C
B1OFF = L1 + L2_ + L3_          # offsets into the 8513-param vector
MASK_BIAS_SHIFT = 2.19

N_CORES = 8
INST_PER_CORE = 4               # 2 pairs
T = 1024                        # positions per chunk
NCHUNK = L // T                 # 40
GROUPS = NCHUNK * 4             # 160 [2, 512] position-groups per core
N_BANKS = (GROUPS + 63) // 64   # 3 output PSUM bank fills (64, 64, 32 groups)

# relu op cost estimates (ns) for greedy ACT/DVE balancing
COST_DVE = (120 + T) / 0.96
COST_ACT = (352 + T) / 1.2

_cache = {}


def _build_program():
    nc = bacc.Bacc("TRN2", target_bir_lowering=False, debug=False)

    xp = nc.dram_tensor("xp", [CIN + 1, L], F32R, kind="ExternalInput")
    w0t = [nc.dram_tensor(f"w0t{p}", [CIN + 1, 128], F32R, kind="ExternalInput")
           for p in range(2)]
    w1t = [nc.dram_tensor(f"w1t{p}", [128, 128], BF, kind="ExternalInput")
           for p in range(2)]
    w2p = [nc.dram_tensor(f"w2p{p}", [128, 512], BF, kind="ExternalInput")
           for p in range(2)]
    b1v = [nc.dram_tensor(f"b1v{p}", [128, 1], F32, kind="ExternalInput")
           for p in range(2)]
    b2v = nc.dram_tensor("b2v", [128, 1], F32, kind="ExternalInput")
    o = nc.dram_tensor("o", [INST_PER_CORE, L], F32, kind="ExternalOutput")

    eng_ns = {"dve": 0.0, "act": 0.0}

    def relu(dst, src, bias_ap):
        if bias_ap is None and eng_ns["dve"] + COST_DVE <= eng_ns["act"] + COST_ACT:
            eng_ns["dve"] += COST_DVE
            if bias_ap is None:
                nc.vector.tensor_scalar(out=dst, in0=src, scalar1=0.0,
                                        scalar2=None, op0=AL.max)
            else:
                nc.vector.tensor_scalar(out=dst, in0=src, scalar1=bias_ap,
                                        scalar2=0.0, op0=AL.add, op1=AL.max)
        else:
            eng_ns["act"] += COST_ACT
            if bias_ap is None:
                nc.scalar.activation(dst, src, AT.Relu)
            else:
                nc.scalar.activation(dst, src, AT.Relu, bias=bias_ap)

    with TileContext(nc) as tc:
        with tc.tile_pool(name="consts", bufs=1) as cpool, \
             tc.tile_pool(name="xpool", bufs=3) as xpool, \
             tc.tile_pool(name="zpool", bufs=3, space="PSUM") as zpool, \
             tc.tile_pool(name="opool", bufs=2, space="PSUM") as opool, \
             tc.tile_pool(name="h1pool", bufs=3) as h1pool, \
             tc.tile_pool(name="h2pool", bufs=3) as h2pool, \
             tc.tile_pool(name="spool", bufs=2) as spool:

            w0_sb, w1_sb, w2_sb, b1_sb = [], [], [], []
            for p in range(2):
                t0 = cpool.tile([CIN + 1, 128], F32R, name=f"w0sb{p}")
                nc.sync.dma_start(out=t0, in_=w0t[p].ap())
                w0_sb.append(t0)
                t1 = cpool.tile([128, 128], BF, name=f"w1sb{p}")
                nc.sync.dma_start(out=t1, in_=w1t[p].ap())
                w1_sb.append(t1)
                t2 = cpool.tile([128, 512], BF, name=f"w2sb{p}")
                nc.sync.dma_start(out=t2, in_=w2p[p].ap())
                w2_sb.append(t2)
                t3 = cpool.tile([128, 1], F32, name=f"b1sb{p}")
                nc.sync.dma_start(out=t3, in_=b1v[p].ap())
                b1_sb.append(t3)
            b2_sb = cpool.tile([128, 1], F32, name="b2sb")
            nc.sync.dma_start(out=b2_sb, in_=b2v.ap())

            feats = {}   # chunk -> tile
            z1s, z2s, h1s, h2s = {}, {}, {}, {}
            obank = {"tile": None, "idx": -1}

            def flush_obank(nrows):
                ob = obank["tile"]
                b = obank["idx"]
                stage = spool.tile([128, 512], F32, name=f"stage{b}", tag="stage")
                nc.vector.tensor_scalar(out=stage[:nrows], in0=ob[:nrows],
                                        scalar1=b2_sb[:nrows, 0:1], scalar2=None,
                                        op0=AL.add)
                ncb = nrows // 8
                # partition q = 8*cb + 4*pair + 2*s + m ; DRAM offset =
                # (2*pair+m)*L + (16*b+cb)*1024 + s*512. One DMA per (pair, s)
                # keeps both APs at <=3 dims.
                src5 = stage.rearrange("(cb pr ss m) f -> cb pr ss m f",
                                       pr=2, ss=2, m=2)
                for pair in range(2):
                    for s in range(2):
                        for mm in range(2):
                            dst = bass.AP(o,
                                          b * 16 * T + (2 * pair + mm) * L + s * 512,
                                          [[T, ncb], [1, 512]])
                            nc.sync.dma_start(out=dst,
                                              in_=src5[:ncb, pair, s, mm, :])

            # software-pipelined emission: iter i does DMA(i+1), L1(i),
            # L3(i-2), L2(i-1); relus follow their producers.
            for i in range(NCHUNK + 2):
                if i == 0:
                    f0 = xpool.tile([CIN + 1, T], F32R, name="feats0", tag="feats")
                    nc.sync.dma_start(out=f0, in_=xp.ap()[:, 0:T])
                    feats[0] = f0
                if i + 1 < NCHUNK:
                    fn = xpool.tile([CIN + 1, T], F32R, name=f"feats{i+1}",
                                    tag="feats")
                    nc.sync.dma_start(out=fn, in_=xp.ap()[:, (i + 1) * T:(i + 2) * T])
                    feats[i + 1] = fn

                # L1(i)
                if i < NCHUNK:
                    for p in range(2):
                        z1 = zpool.tile([128, T], F32, name=f"z1_{i}_{p}", tag="z")
                        for s in range(2):
                            nc.tensor.matmul(z1[:, s * 512:(s + 1) * 512], w0_sb[p],
                                             feats[i][:, s * 512:(s + 1) * 512],
                                             start=True, stop=True)
                        z1s[(i, p)] = z1
                    for p in range(2):
                        h1 = h1pool.tile([128, T], BF, name=f"h1_{i}_{p}", tag="h1")
                        relu(h1, z1s.pop((i, p)), None)
                        h1s[(i, p)] = h1
                    feats.pop(i)

                # L3(i-2)
                j3 = i - 2
                if j3 >= 0:
                    for p in range(2):
                        h2 = h2s[(j3, p)]
                        for s in range(2):
                            g = j3 * 4 + p * 2 + s
                            lg = g % 64
                            if lg == 0:
                                obank["tile"] = opool.tile([128, 512], F32,
                                                           name=f"ob{g//64}",
                                                           tag="ob")
                                obank["idx"] = g // 64
                            jj, jv = lg // 16, lg % 16
                            nc.tensor.matmul(
                                obank["tile"][32 * jj:32 * jj + 32, :],
                                w2_sb[p][:, 32 * jv:32 * jv + 32],
                                h2[:, s * 512:(s + 1) * 512],
                                start=(jv == 0), stop=(jv == 15),
                                tile_position=(0, 32 * jj))
                            if g == GROUPS - 1:
                                flush_obank(((g % 64) + 1) * 2)
                            elif lg == 63:
                                flush_obank(128)
                        if j3 >= 1:
                            h2s.pop((j3 - 1, p), None)

                # L2(i-1)
                j2 = i - 1
                if 0 <= j2 < NCHUNK:
                    for p in range(2):
                        z2 = zpool.tile([128, T], F32, name=f"z2_{j2}_{p}", tag="z")
                        h1 = h1s.pop((j2, p))
                        for s in range(2):
                            nc.tensor.matmul(z2[:, s * 512:(s + 1) * 512], w1_sb[p],
                                             h1[:, s * 512:(s + 1) * 512],
                                             start=True, stop=True)
                        z2s[(j2, p)] = z2
                    for p in range(2):
                        h2 = h2pool.tile([128, T], BF, name=f"h2_{j2}_{p}", tag="h2")
                        relu(h2, z2s.pop((j2, p)), b1_sb[p][:, 0:1])
                        h2s[(j2, p)] = h2

    nc.compile()
    return nc


def _prep_inputs(x, mask_head_params, num_ins):
    x = np.asarray(x, dtype=np.float32)
    params = np.asarray(mask_head_params, dtype=np.float32)
    num_ins = np.asarray(num_ins)
    img_idx = np.repeat(np.arange(N_IMG), num_ins)
    assert img_idx.shape[0] == N_IMG * INS_PER_IMG

    # xplus per image: [locx; locy; ones; x]  -> [67, L] fp32
    loc_x = np.broadcast_to(np.arange(W, dtype=np.float32)[None, :], (H, W))
    loc_y = np.broadcast_to(np.arange(H, dtype=np.float32)[:, None], (H, W))
    xplus = np.empty((N_IMG, CIN + 1, L), dtype=np.float32)
    for img in range(N_IMG):
        xplus[img, 0] = loc_x.reshape(L)
        xplus[img, 1] = loc_y.reshape(L)
        xplus[img, 2] = 1.0
        xplus[img, 3:] = x[img].reshape(C, L)

    in_maps = []
    for c in range(N_CORES):
        inst = [4 * c + k for k in range(INST_PER_CORE)]
        imgs = {img_idx[q] for q in inst}
        assert len(imgs) == 1, "expected each core's instances on one image"
        m = {"xp": xplus[img_idx[inst[0]]]}
        for p in range(2):
            a, b = inst[2 * p], inst[2 * p + 1]
            w0_a = params[a, :L1].reshape(C, CIN)
            w0_b = params[b, :L1].reshape(C, CIN)
            b0_a = params[a, B1OFF:B1OFF + C]
            b0_b = params[b, B1OFF:B1OFF + C]
            # lhsT rows: [w0[:,0]; w0[:,1]; b0; w0[:,2:].T] per instance col blk
            w0t = np.zeros((CIN + 1, 128), np.float32)
            for k, (wv, bv) in enumerate(((w0_a, b0_a), (w0_b, b0_b))):
                cols = slice(64 * k, 64 * k + 64)
                w0t[0, cols] = wv[:, 0]
                w0t[1, cols] = wv[:, 1]
                w0t[2, cols] = bv
                w0t[3:, cols] = wv[:, 2:].T
            m[f"w0t{p}"] = w0t

            w1_a = params[a, L1:L1 + L2_].reshape(C, C)
            w1_b = params[b, L1:L1 + L2_].reshape(C, C)
            w1t = np.zeros((128, 128), np.float32)
            w1t[:64, :64] = w1_a.T
            w1t[64:, 64:] = w1_b.T
            m[f"w1t{p}"] = w1t.astype(bf16)

            w2_a = params[a, L1 + L2_:L1 + L2_ + C]
            w2_b = params[b, L1 + L2_:L1 + L2_ + C]
            w2pair = np.zeros((128, 2), np.float32)
            w2pair[:64, 0] = w2_a
            w2pair[64:, 1] = w2_b
            w2pad = np.zeros((128, 512), np.float32)
            for j in range(16):
                w2pad[:, 32 * j + 2 * j:32 * j + 2 * j + 2] = w2pair
            m[f"w2p{p}"] = w2pad.astype(bf16)

            b1 = np.concatenate([params[a, B1OFF + C:B1OFF + 2 * C],
                                 params[b, B1OFF + C:B1OFF + 2 * C]])
            m[f"b1v{p}"] = b1.reshape(128, 1).astype(np.float32)

        # b2 per out-bank partition q: pair=((q//2)%4)//2, inst_in_pair=q%2
        b2 = np.empty((128, 1), np.float32)
        for q in range(128):
            pair = ((q // 2) % 4) // 2
            mm = q % 2
            iid = inst[2 * pair + mm]
            b2[q, 0] = params[iid, B1OFF + 2 * C] - MASK_BIAS_SHIFT
        m["b2v"] = b2
        in_maps.append(m)
    return in_maps


def kernel(x, mask_head_params, num_ins):
    if "nc" not in _cache:
        _cache["nc"] = _build_program()
    nc = _cache["nc"]
    in_maps = _prep_inputs(x, mask_head_params, num_ins)
    res = run_bass_kernel_spmd(nc, in_maps, core_ids=list(range(N_CORES)))
    out = np.concatenate([r["o"] for r in res.results], axis=0)  # [32, L]
    return out.reshape(1, N_IMG * INS_PER_IMG, H, W).astype(np.float32)



# revision 7
# speedup vs baseline: 19903.1698x; 19903.1698x over previous
"""CondLaneHead DynamicMaskHead kernel for 8 Trainium2 NeuronCores.

Problem: per-instance 3-layer 1x1-conv MLP over a [64,160,256] feature map.
  feats = concat([loc_x, loc_y], x[img])            # [66, L], L = 160*256
  h1 = relu(w0 @ feats + b0)                        # [64, L]
  h2 = relu(w1 @ h1 + b1)                           # [64, L]
  out = w2 @ h2 + b2 - 2.19                         # [1, L]
32 instances (8 per image, 4 images). Sharding: 4 instances per core; each
core needs exactly one image's feature map.

Device mapping (per core):
  - The 4 instances form 2 pairs. Layer 1: one matmul per pair with
    lhsT = [67, 128] (2 coord rows + ones row for the bias + 64 x rows).
    Layer 2: block-diagonal [128, 128] weights, one matmul per pair.
  - Layer 3 (64->1): output is packed across PSUM partitions. Matmuls write
    [32, 512] windows at partition bases 0/32/64/96 using zero-padded weight
    variants (w2 placed at columns 2j of window j), accumulating over 16
    position-groups per window, so one PSUM bank collects 64 groups x 2
    instances = a full [128, 512] tile before a single bias+copy op and one
    scatter-DMA to DRAM.
  - dtypes: layer 1 runs in float32r (full-rate fp32 storage on the PE),
    h1/h2 and layer-2/3 weights in bf16, all PSUM accumulation in fp32.
"""

import sys

if "/opt/trn_rl_repo" not in sys.path:
    sys.path.insert(0, "/opt/trn_rl_repo")

import numpy as np
import ml_dtypes

import concourse.bass as bass
import concourse.mybir as mybir
from concourse import bacc
from concourse.tile import TileContext
from concourse.bass_utils import run_bass_kernel_spmd

BF = mybir.dt.bfloat16
F32 = mybir.dt.float32
F32R = mybir.dt.float32r
AT = mybir.ActivationFunctionType
AL = mybir.AluOpType
bf16 = ml_dtypes.bfloat16

# Problem geometry (hardcoded per spec)
N_IMG, INS_PER_IMG, C, H, W = 4, 8, 64, 160, 256
CIN = C + 2
L = H * W                       # 40960 positions
L1, L2_, L3_ = (C + 2) * C, C * C, C
B1OFF = L1 + L2_ + L3_          # offsets into the 8513-param vector
MASK_BIAS_SHIFT = 2.19

N_CORES = 8
INST_PER_CORE = 4               # 2 pairs
T = 1024                        # positions per chunk
NCHUNK = L // T                 # 40
GROUPS = NCHUNK * 4             # 160 [2, 512] position-groups per core
N_BANKS = (GROUPS + 63) // 64   # 3 output PSUM bank fills (64, 64, 32 groups)

# relu op cost estimates (ns) for greedy ACT/DVE balancing
COST_DVE = (120 + T) / 0.96
COST_ACT = (352 + T) / 1.2

_cache = {}


def _build_program():
    nc = bacc.Bacc("TRN2", target_bir_lowering=False, debug=False)

    xp = nc.dram_tensor("xp", [CIN + 2, L], F32R, kind="ExternalInput")
    w0t = [nc.dram_tensor(f"w0t{p}", [CIN + 2, 128], F32R, kind="ExternalInput")
           for p in range(2)]
    w1t = [nc.dram_tensor(f"w1t{p}", [128, 128], BF, kind="ExternalInput")
           for p in range(2)]
    w2p = [nc.dram_tensor(f"w2p{p}", [128, 512], BF, kind="ExternalInput")
           for p in range(2)]
    b1v = [nc.dram_tensor(f"b1v{p}", [128, 1], F32, kind="ExternalInput")
           for p in range(2)]
    b2v = nc.dram_tensor("b2v", [128, 1], F32, kind="ExternalInput")
    o = nc.dram_tensor("o", [INST_PER_CORE, L], F32, kind="ExternalOutput")

    eng_ns = {"dve": 0.0, "act": 0.0}

    def relu(dst, src, bias_ap):
        if bias_ap is None and eng_ns["dve"] + COST_DVE <= eng_ns["act"] + COST_ACT:
            eng_ns["dve"] += COST_DVE
            if bias_ap is None:
                nc.vector.tensor_scalar(out=dst, in0=src, scalar1=0.0,
                                        scalar2=None, op0=AL.max)
            else:
                nc.vector.tensor_scalar(out=dst, in0=src, scalar1=bias_ap,
                                        scalar2=0.0, op0=AL.add, op1=AL.max)
        else:
            eng_ns["act"] += COST_ACT
            if bias_ap is None:
                nc.scalar.activation(dst, src, AT.Relu)
            else:
                nc.scalar.activation(dst, src, AT.Relu, bias=bias_ap)

    with TileContext(nc) as tc:
        with tc.tile_pool(name="consts", bufs=1) as cpool, \
             tc.tile_pool(name="xpool", bufs=3) as xpool, \
             tc.tile_pool(name="zpool", bufs=3, space="PSUM") as zpool, \
             tc.tile_pool(name="opool", bufs=2, space="PSUM") as opool, \
             tc.tile_pool(name="h1pool", bufs=3) as h1pool, \
             tc.tile_pool(name="h2pool", bufs=3) as h2pool, \
             tc.tile_pool(name="spool", bufs=2) as spool:

            w0_sb, w1_sb, w2_sb, b1_sb = [], [], [], []
            for p in range(2):
                t0 = cpool.tile([CIN + 2, 128], F32R, name=f"w0sb{p}")
                nc.sync.dma_start(out=t0, in_=w0t[p].ap())
                w0_sb.append(t0)
                t1 = cpool.tile([128, 128], BF, name=f"w1sb{p}")
                nc.sync.dma_start(out=t1, in_=w1t[p].ap())
                w1_sb.append(t1)
                t2 = cpool.tile([128, 512], BF, name=f"w2sb{p}")
                nc.sync.dma_start(out=t2, in_=w2p[p].ap())
                w2_sb.append(t2)
                t3 = cpool.tile([128, 1], F32, name=f"b1sb{p}")
                nc.sync.dma_start(out=t3, in_=b1v[p].ap())
                b1_sb.append(t3)
            b2_sb = cpool.tile([128, 1], F32, name="b2sb")
            nc.sync.dma_start(out=b2_sb, in_=b2v.ap())

            feats = {}   # chunk -> tile
            z1s, z2s, h1s, h2s = {}, {}, {}, {}
            obank = {"tile": None, "idx": -1}

            def flush_obank(nrows):
                ob = obank["tile"]
                b = obank["idx"]
                stage = spool.tile([128, 512], F32, name=f"stage{b}", tag="stage")
                nc.vector.tensor_scalar(out=stage[:nrows], in0=ob[:nrows],
                                        scalar1=b2_sb[:nrows, 0:1], scalar2=None,
                                        op0=AL.add)
                ncb = nrows // 8
                # partition q = 8*cb + 4*pair + 2*s + m ; DRAM offset =
                # (2*pair+m)*L + (16*b+cb)*1024 + s*512. One DMA per (pair, s)
                # keeps both APs at <=3 dims.
                src5 = stage.rearrange("(cb pr ss m) f -> cb pr ss m f",
                                       pr=2, ss=2, m=2)
                for pair in range(2):
                    for s in range(2):
                        for mm in range(2):
                            dst = bass.AP(o,
                                          b * 16 * T + (2 * pair + mm) * L + s * 512,
                                          [[T, ncb], [1, 512]])
                            nc.sync.dma_start(out=dst,
                                              in_=src5[:ncb, pair, s, mm, :])

            # software-pipelined emission: iter i does DMA(i+1), L1(i),
            # L3(i-2), L2(i-1); relus follow their producers.
            for i in range(NCHUNK + 2):
                if i == 0:
                    f0 = xpool.tile([CIN + 2, T], F32R, name="feats0", tag="feats")
                    nc.sync.dma_start(out=f0, in_=xp.ap()[:, 0:T])
                    feats[0] = f0
                if i + 1 < NCHUNK:
                    fn = xpool.tile([CIN + 2, T], F32R, name=f"feats{i+1}",
                                    tag="feats")
                    nc.sync.dma_start(out=fn, in_=xp.ap()[:, (i + 1) * T:(i + 2) * T])
                    feats[i + 1] = fn

                # L1(i)
                if i < NCHUNK:
                    for p in range(2):
                        z1 = zpool.tile([128, T], F32, name=f"z1_{i}_{p}", tag="z")
                        for s in range(2):
                            nc.tensor.matmul(z1[:, s * 512:(s + 1) * 512], w0_sb[p],
                                             feats[i][:, s * 512:(s + 1) * 512],
                                             start=True, stop=True)
                        z1s[(i, p)] = z1
                    for p in range(2):
                        h1 = h1pool.tile([128, T], BF, name=f"h1_{i}_{p}", tag="h1")
                        relu(h1, z1s.pop((i, p)), None)
                        h1s[(i, p)] = h1
                    feats.pop(i)

                # L3(i-2)
                j3 = i - 2
                if j3 >= 0:
                    for p in range(2):
                        h2 = h2s[(j3, p)]
                        for s in range(2):
                            g = j3 * 4 + p * 2 + s
                            lg = g % 64
                            if lg == 0:
                                obank["tile"] = opool.tile([128, 512], F32,
                                                           name=f"ob{g//64}",
                                                           tag="ob")
                                obank["idx"] = g // 64
                            jj, jv = lg // 16, lg % 16
                            nc.tensor.matmul(
                                obank["tile"][32 * jj:32 * jj + 32, :],
                                w2_sb[p][:, 32 * jv:32 * jv + 32],
                                h2[:, s * 512:(s + 1) * 512],
                                start=(jv == 0), stop=(jv == 15),
                                tile_position=(0, 32 * jj))
                            if g == GROUPS - 1:
                                flush_obank(((g % 64) + 1) * 2)
                            elif lg == 63:
                                flush_obank(128)
                        if j3 >= 1:
                            h2s.pop((j3 - 1, p), None)

                # L2(i-1)
                j2 = i - 1
                if 0 <= j2 < NCHUNK:
                    for p in range(2):
                        z2 = zpool.tile([128, T], F32, name=f"z2_{j2}_{p}", tag="z")
                        h1 = h1s.pop((j2, p))
                        for s in range(2):
                            nc.tensor.matmul(z2[:, s * 512:(s + 1) * 512], w1_sb[p],
                                             h1[:, s * 512:(s + 1) * 512],
                                             start=True, stop=True)
                        z2s[(j2, p)] = z2
                    for p in range(2):
                        h2 = h2pool.tile([128, T], BF, name=f"h2_{j2}_{p}", tag="h2")
                        relu(h2, z2s.pop((j2, p)), b1_sb[p][:, 0:1])
                        h2s[(j2, p)] = h2

    nc.compile()
    return nc


def _prep_inputs(x, mask_head_params, num_ins):
    x = np.asarray(x, dtype=np.float32)
    params = np.asarray(mask_head_params, dtype=np.float32)
    num_ins = np.asarray(num_ins)
    img_idx = np.repeat(np.arange(N_IMG), num_ins)
    assert img_idx.shape[0] == N_IMG * INS_PER_IMG

    # xplus per image: [locx; locy; ones; x]  -> [67, L] fp32
    loc_x = np.broadcast_to(np.arange(W, dtype=np.float32)[None, :], (H, W))
    loc_y = np.broadcast_to(np.arange(H, dtype=np.float32)[:, None], (H, W))
    # 68 rows (even partition count): odd-partition DMAs serialize onto a
    # single SDMA engine (~26 GB/s); even counts spread across all 16.
    xplus = np.zeros((N_IMG, CIN + 2, L), dtype=np.float32)
    for img in range(N_IMG):
        xplus[img, 0] = loc_x.reshape(L)
        xplus[img, 1] = loc_y.reshape(L)
        xplus[img, 2] = 1.0
        xplus[img, 3:CIN + 1] = x[img].reshape(C, L)

    in_maps = []
    for c in range(N_CORES):
        inst = [4 * c + k for k in range(INST_PER_CORE)]
        imgs = {img_idx[q] for q in inst}
        assert len(imgs) == 1, "expected each core's instances on one image"
        m = {"xp": xplus[img_idx[inst[0]]]}
        for p in range(2):
            a, b = inst[2 * p], inst[2 * p + 1]
            w0_a = params[a, :L1].reshape(C, CIN)
            w0_b = params[b, :L1].reshape(C, CIN)
            b0_a = params[a, B1OFF:B1OFF + C]
            b0_b = params[b, B1OFF:B1OFF + C]
            # lhsT rows: [w0[:,0]; w0[:,1]; b0; w0[:,2:].T] per instance col blk
            w0t = np.zeros((CIN + 2, 128), np.float32)
            for k, (wv, bv) in enumerate(((w0_a, b0_a), (w0_b, b0_b))):
                cols = slice(64 * k, 64 * k + 64)
                w0t[0, cols] = wv[:, 0]
                w0t[1, cols] = wv[:, 1]
                w0t[2, cols] = bv
                w0t[3:CIN + 1, cols] = wv[:, 2:].T
            m[f"w0t{p}"] = w0t

            w1_a = params[a, L1:L1 + L2_].reshape(C, C)
            w1_b = params[b, L1:L1 + L2_].reshape(C, C)
            w1t = np.zeros((128, 128), np.float32)
            w1t[:64, :64] = w1_a.T
            w1t[64:, 64:] = w1_b.T
            m[f"w1t{p}"] = w1t.astype(bf16)

            w2_a = params[a, L1 + L2_:L1 + L2_ + C]
            w2_b = params[b, L1 + L2_:L1 + L2_ + C]
            w2pair = np.zeros((128, 2), np.float32)
            w2pair[:64, 0] = w2_a
            w2pair[64:, 1] = w2_b
            w2pad = np.zeros((128, 512), np.float32)
            for j in range(16):
                w2pad[:, 32 * j + 2 * j:32 * j + 2 * j + 2] = w2pair
            m[f"w2p{p}"] = w2pad.astype(bf16)

            b1 = np.concatenate([params[a, B1OFF + C:B1OFF + 2 * C],
                                 params[b, B1OFF + C:B1OFF + 2 * C]])
            m[f"b1v{p}"] = b1.reshape(128, 1).astype(np.float32)

        # b2 per out-bank partition q: pair=((q//2)%4)//2, inst_in_pair=q%2
        b2 = np.empty((128, 1), np.float32)
        for q in range(128):
            pair = ((q // 2) % 4) // 2
            mm = q % 2
            iid = inst[2 * pair + mm]
            b2[q, 0] = params[iid, B1OFF + 2 * C] - MASK_BIAS_SHIFT
        m["b2v"] = b2
        in_maps.append(m)
    return in_maps


def kernel(x, mask_head_params, num_ins):
    if "nc" not in _cache:
        _cache["nc"] = _build_program()
    nc = _cache["nc"]
    in_maps = _prep_inputs(x, mask_head_params, num_ins)
    res = run_bass_kernel_spmd(nc, in_maps, core_ids=list(range(N_CORES)))
    out = np.concatenate([r["o"] for r in res.results], axis=0)  # [32, L]
    return out.reshape(1, N_IMG * INS_PER_IMG, H, W).astype(np.float32)



# revision 13
# speedup vs baseline: 21129.2311x; 1.0616x over previous
"""CondLaneHead DynamicMaskHead kernel for 8 Trainium2 NeuronCores.

Problem: per-instance 3-layer 1x1-conv MLP over a [64,160,256] feature map.
  feats = concat([loc_x, loc_y], x[img])            # [66, L], L = 160*256
  h1 = relu(w0 @ feats + b0)                        # [64, L]
  h2 = relu(w1 @ h1 + b1)                           # [64, L]
  out = w2 @ h2 + b2 - 2.19                         # [1, L]
32 instances (8 per image, 4 images).

Sharding (hybrid): core c -> image c//2, position half c%2. Each core runs
all 8 instances of its image over Lc = L/2 = 20480 positions, so every byte
of x is shipped to exactly one core.

Device mapping (per core, 8 instances = 4 pairs, 20 chunks of T=1024):
  - feats live in 10 resident SBUF tiles [66, 2048] bf16: rows 0-63 = x
    chunk (64-row DMA -> spreads across all 16 SDMA engines; odd partition
    counts serialize onto one engine at ~26 GB/s), row 64 = loc_x pattern,
    row 65 = (col//256) pattern (both periodic with 1024, loaded once).
    The chunk-dependent part of the loc_y term, w0y*(80*half + 4*i), is
    folded into a per-chunk L1 relu bias together with b0.
  - L1: per pair one [66,128] bf16 lhsT (2 instances side by side), two
    512-col matmuls per chunk into a [128,1024] PSUM tile.
  - L2: block-diagonal [128,128] bf16 weights per pair.
  - L3 (64->1): outputs packed across PSUM partitions: per 512-position
    group one matmul with a zero-padded [128,32] w2 slice writes a [32,512]
    window (tile_position), accumulating 16 groups per window, so one
    [128,512] PSUM bank collects 64 groups before a single bias+copy op
    and a few strided DMAs to DRAM.
  - relu work (PSUM->SBUF copies) is split greedily between ACT and DVE.
"""

import sys

if "/opt/trn_rl_repo" not in sys.path:
    sys.path.insert(0, "/opt/trn_rl_repo")

import numpy as np
import ml_dtypes

import concourse.bass as bass
import concourse.mybir as mybir
from concourse import bacc
from concourse.tile import TileContext
from concourse.bass_utils import run_bass_kernel_spmd

BF = mybir.dt.bfloat16
F32 = mybir.dt.float32
AT = mybir.ActivationFunctionType
AL = mybir.AluOpType
bf16 = ml_dtypes.bfloat16

# Problem geometry (hardcoded per spec)
N_IMG, INS_PER_IMG, C, H, W = 4, 8, 64, 160, 256
CIN = C + 2
L = H * W                       # 40960 positions per image
L1, L2_, L3_ = (C + 2) * C, C * C, C
B1OFF = L1 + L2_ + L3_          # offsets into the 8513-param vector
MASK_BIAS_SHIFT = 2.19

N_CORES = 8
NPAIR = 4                       # 8 instances per core, 2 per matmul pack
LC = L // 2                     # 20480 positions per core
T = 1024                        # positions per chunk
NCHUNK = LC // T                # 20
FT = 2048                       # positions per feats SBUF tile
NFT = LC // FT                  # 10 resident feats tiles
GROUPS = NCHUNK * NPAIR * 2     # 160 [2, 512] position-groups per core
N_BANKS = (GROUPS + 63) // 64   # 3 output PSUM bank fills (64, 64, 32)

# relu op cost estimates (ns) for greedy ACT/DVE balancing
COST_DVE = (120 + T) / 0.96
COST_ACT = (352 + T) / 1.2

_cache = {}


def _build_program():
    nc = bacc.Bacc("TRN2", target_bir_lowering=False, debug=False)

    xb = nc.dram_tensor("xb", [C, LC], BF, kind="ExternalInput")
    coords = nc.dram_tensor("coords", [2, FT], BF, kind="ExternalInput")
    w0t = [nc.dram_tensor(f"w0t{p}", [CIN, 128], BF, kind="ExternalInput")
           for p in range(NPAIR)]
    w1t = [nc.dram_tensor(f"w1t{p}", [128, 128], BF, kind="ExternalInput")
           for p in range(NPAIR)]
    w2p = [nc.dram_tensor(f"w2p{p}", [128, 512], BF, kind="ExternalInput")
           for p in range(NPAIR)]
    b0v = [nc.dram_tensor(f"b0v{p}", [128, NCHUNK], F32, kind="ExternalInput")
           for p in range(NPAIR)]
    b1v = [nc.dram_tensor(f"b1v{p}", [128, 1], F32, kind="ExternalInput")
           for p in range(NPAIR)]
    b2v = nc.dram_tensor("b2v", [128, 1], F32, kind="ExternalInput")
    o = nc.dram_tensor("o", [2 * NPAIR, LC], F32, kind="ExternalOutput")

    eng_ns = {"dve": 0.0, "act": 0.0}

    def relu(dst, src, bias_ap):
        if eng_ns["dve"] + COST_DVE <= eng_ns["act"] + COST_ACT:
            eng_ns["dve"] += COST_DVE
            if bias_ap is None:
                nc.vector.tensor_scalar(out=dst, in0=src, scalar1=0.0,
                                        scalar2=None, op0=AL.max)
            else:
                nc.vector.tensor_scalar(out=dst, in0=src, scalar1=bias_ap,
                                        scalar2=0.0, op0=AL.add, op1=AL.max)
        else:
            eng_ns["act"] += COST_ACT
            if bias_ap is None:
                nc.scalar.activation(dst, src, AT.Relu)
            else:
                nc.scalar.activation(dst, src, AT.Relu, bias=bias_ap)

    with TileContext(nc) as tc:
        with tc.tile_pool(name="consts", bufs=1) as cpool, \
             tc.tile_pool(name="zpool", bufs=3, space="PSUM") as zpool, \
             tc.tile_pool(name="opool", bufs=2, space="PSUM") as opool, \
             tc.tile_pool(name="h1pool", bufs=10) as h1pool, \
             tc.tile_pool(name="h2pool", bufs=10) as h2pool, \
             tc.tile_pool(name="spool", bufs=2) as spool:

            # resident feats tiles: x rows via 64-partition DMAs, coord rows
            # (periodic patterns, identical for every tile) via tiny DMAs
            fts = []
            for j in range(NFT):
                ft = cpool.tile([CIN, FT], BF, name=f"ft{j}")
                nc.sync.dma_start(out=ft[0:C, :],
                                  in_=xb.ap()[:, j * FT:(j + 1) * FT])
                nc.sync.dma_start(out=ft[C:CIN, :], in_=coords.ap())
                fts.append(ft)

            w0_sb, w1_sb, w2_sb, b0_sb, b1_sb = [], [], [], [], []
            for p in range(NPAIR):
                t0 = cpool.tile([CIN, 128], BF, name=f"w0sb{p}")
                nc.sync.dma_start(out=t0, in_=w0t[p].ap())
                w0_sb.append(t0)
                t1 = cpool.tile([128, 128], BF, name=f"w1sb{p}")
                nc.sync.dma_start(out=t1, in_=w1t[p].ap())
                w1_sb.append(t1)
                t2 = cpool.tile([128, 512], BF, name=f"w2sb{p}")
                nc.sync.dma_start(out=t2, in_=w2p[p].ap())
                w2_sb.append(t2)
                t3 = cpool.tile([128, NCHUNK], F32, name=f"b0sb{p}")
                nc.sync.dma_start(out=t3, in_=b0v[p].ap())
                b0_sb.append(t3)
                t4 = cpool.tile([128, 1], F32, name=f"b1sb{p}")
                nc.sync.dma_start(out=t4, in_=b1v[p].ap())
                b1_sb.append(t4)
            b2_sb = cpool.tile([128, 1], F32, name="b2sb")
            nc.sync.dma_start(out=b2_sb, in_=b2v.ap())

            z1s, z2s, h1s, h2s = {}, {}, {}, {}
            obank = {"tile": None, "idx": -1}

            def flush_obank(nrows):
                ob = obank["tile"]
                b = obank["idx"]
                stage = spool.tile([128, 512], F32, name=f"stage{b}",
                                   tag="stage")
                nc.vector.tensor_scalar(out=stage[:nrows], in0=ob[:nrows],
                                        scalar1=b2_sb[:nrows, 0:1],
                                        scalar2=None, op0=AL.add)
                na = nrows // 32
                # partition q = 32a + 16bb + (4p + 2s + m) for chunk
                # i = 8b + 2a + bb; DRAM offset = (2p+m)*LC + i*1024 + s*512.
                # Only an AP's first dim is the partition dim, so each DMA
                # fixes (bb, 4p+2s+m) and spans a over partitions.
                src = stage.rearrange("(a bb c) f -> a bb c f", bb=2, c=16)
                for p in range(NPAIR):
                    for s in range(2):
                        for m in range(2):
                            for bb in range(2):
                                dst = bass.AP(
                                    o,
                                    (2 * p + m) * LC + b * 8 * T
                                    + bb * T + s * 512,
                                    [[2 * T, na], [1, 512]])
                                nc.sync.dma_start(
                                    out=dst,
                                    in_=src[:na, bb, 4 * p + 2 * s + m, :])

            # software-pipelined emission: iter i does L1(i), L3(i-2),
            # L2(i-1); relus follow their producers.
            for i in range(NCHUNK + 2):
                # L1(i)
                if i < NCHUNK:
                    ft = fts[i // 2]
                    c0 = (i % 2) * T
                    for p in range(NPAIR):
                        z1 = zpool.tile([128, T], F32, name=f"z1_{i}_{p}",
                                        tag="z")
                        for s in range(2):
                            nc.tensor.matmul(
                                z1[:, s * 512:(s + 1) * 512], w0_sb[p],
                                ft[:, c0 + s * 512:c0 + (s + 1) * 512],
                                start=True, stop=True)
                        h1 = h1pool.tile([128, T], BF, name=f"h1_{i}_{p}",
                                         tag="h1")
                        relu(h1, z1, b0_sb[p][:, i:i + 1])
                        h1s[(i, p)] = h1

                # L3(i-2)
                j3 = i - 2
                if j3 >= 0:
                    for p in range(NPAIR):
                        h2 = h2s[(j3, p)]
                        for s in range(2):
                            g = j3 * 8 + p * 2 + s
                            lg = g % 64
                            if lg == 0:
                                obank["tile"] = opool.tile(
                                    [128, 512], F32, name=f"ob{g // 64}",
                                    tag="ob")
                                obank["idx"] = g // 64
                            jj, jv = lg // 16, lg % 16
                            nc.tensor.matmul(
                                obank["tile"][32 * jj:32 * jj + 32, :],
                                w2_sb[p][:, 32 * jv:32 * jv + 32],
                                h2[:, s * 512:(s + 1) * 512],
                                start=(jv == 0), stop=(jv == 15),
                                tile_position=(0, 32 * jj))
                            if g == GROUPS - 1:
                                flush_obank(((g % 64) + 1) * 2)
                            elif lg == 63:
                                flush_obank(128)
                        if j3 >= 1:
                            h2s.pop((j3 - 1, p), None)

                # L2(i-1)
                j2 = i - 1
                if 0 <= j2 < NCHUNK:
                    for p in range(NPAIR):
                        z2 = zpool.tile([128, T], F32, name=f"z2_{j2}_{p}",
                                        tag="z")
                        h1 = h1s.pop((j2, p))
                        for s in range(2):
                            nc.tensor.matmul(z2[:, s * 512:(s + 1) * 512],
                                             w1_sb[p],
                                             h1[:, s * 512:(s + 1) * 512],
                                             start=True, stop=True)
                        h2 = h2pool.tile([128, T], BF, name=f"h2_{j2}_{p}",
                                         tag="h2")
                        relu(h2, z2, b1_sb[p][:, 0:1])
                        h2s[(j2, p)] = h2

    nc.compile()
    return nc


def _prep_inputs(x, mask_head_params, num_ins):
    x = np.asarray(x, dtype=np.float32)
    params = np.asarray(mask_head_params, dtype=np.float32)
    num_ins = np.asarray(num_ins)
    img_idx = np.repeat(np.arange(N_IMG), num_ins)
    assert img_idx.shape[0] == N_IMG * INS_PER_IMG

    xbf = x.reshape(N_IMG, C, L).astype(bf16)

    # coord rows, periodic with T=1024 so one [2, FT] block serves all tiles:
    # row 0 = loc_x = col % 256; row 1 = (col // 256) % 4 (loc_y base).
    cols = np.arange(FT)
    coords = np.stack([cols % W, (cols // W) % 4]).astype(bf16)

    in_maps = []
    for c in range(N_CORES):
        img, half = c // 2, c % 2
        inst = [img * INS_PER_IMG + k for k in range(INS_PER_IMG)]
        m = {"xb": np.ascontiguousarray(
                 xbf[img][:, half * LC:(half + 1) * LC]),
             "coords": coords}
        for p in range(NPAIR):
            a, b = inst[2 * p], inst[2 * p + 1]
            w0_a = params[a, :L1].reshape(C, CIN)
            w0_b = params[b, :L1].reshape(C, CIN)
            # lhsT rows: [w0[:,2:].T ; w0[:,0] (loc_x) ; w0[:,1] (loc_y)]
            w0tp = np.zeros((CIN, 128), np.float32)
            for k, wv in enumerate((w0_a, w0_b)):
                colsl = slice(64 * k, 64 * k + 64)
                w0tp[0:C, colsl] = wv[:, 2:].T
                w0tp[C, colsl] = wv[:, 0]
                w0tp[C + 1, colsl] = wv[:, 1]
            m[f"w0t{p}"] = w0tp.astype(bf16)

            # per-chunk L1 bias: b0 + w0y * (80*half + 4*i)
            b0pair = np.concatenate([params[a, B1OFF:B1OFF + C],
                                     params[b, B1OFF:B1OFF + C]])
            w0y = np.concatenate([w0_a[:, 1], w0_b[:, 1]])
            ii = np.arange(NCHUNK, dtype=np.float32)
            m[f"b0v{p}"] = (b0pair[:, None] +
                            w0y[:, None] * (80.0 * half + 4.0 * ii[None, :])
                            ).astype(np.float32)

            w1_a = params[a, L1:L1 + L2_].reshape(C, C)
            w1_b = params[b, L1:L1 + L2_].reshape(C, C)
            w1tp = np.zeros((128, 128), np.float32)
            w1tp[:64, :64] = w1_a.T
            w1tp[64:, 64:] = w1_b.T
            m[f"w1t{p}"] = w1tp.astype(bf16)

            w2_a = params[a, L1 + L2_:L1 + L2_ + C]
            w2_b = params[b, L1 + L2_:L1 + L2_ + C]
            w2pair = np.zeros((128, 2), np.float32)
            w2pair[:64, 0] = w2_a
            w2pair[64:, 1] = w2_b
            w2pad = np.zeros((128, 512), np.float32)
            for j in range(16):
                w2pad[:, 32 * j + 2 * j:32 * j + 2 * j + 2] = w2pair
            m[f"w2p{p}"] = w2pad.astype(bf16)

            b1 = np.concatenate([params[a, B1OFF + C:B1OFF + 2 * C],
                                 params[b, B1OFF + C:B1OFF + 2 * C]])
            m[f"b1v{p}"] = b1.reshape(128, 1).astype(np.float32)

        # b2 per out-bank partition q = 32a + 16bb + (4p + 2s + m):
        # instance = 2p + m with p = (q%16)//4, m = q%2
        b2 = np.empty((128, 1), np.float32)
        for q in range(128):
            p = (q % 16) // 4
            mm = q % 2
            iid = inst[2 * p + mm]
            b2[q, 0] = params[iid, B1OFF + 2 * C] - MASK_BIAS_SHIFT
        m["b2v"] = b2
        in_maps.append(m)
    return in_maps


def kernel(x, mask_head_params, num_ins):
    if "nc" not in _cache:
        _cache["nc"] = _build_program()
    nc = _cache["nc"]
    in_maps = _prep_inputs(x, mask_head_params, num_ins)
    res = run_bass_kernel_spmd(nc, in_maps, core_ids=list(range(N_CORES)))
    out = np.empty((N_IMG * INS_PER_IMG, L), dtype=np.float32)
    for c in range(N_CORES):
        img, half = c // 2, c % 2
        out[img * INS_PER_IMG:(img + 1) * INS_PER_IMG,
            half * LC:(half + 1) * LC] = res.results[c]["o"]
    return out.reshape(1, N_IMG * INS_PER_IMG, H, W).astype(np.float32)


# revision 17
# speedup vs baseline: 23740.4627x; 1.1236x over previous
"""CondLaneHead DynamicMaskHead kernel for 8 Trainium2 NeuronCores.

Problem: per-instance 3-layer 1x1-conv MLP over a [64,160,256] feature map.
  feats = concat([loc_x, loc_y], x[img])            # [66, L], L = 160*256
  h1 = relu(w0 @ feats + b0)                        # [64, L]
  h2 = relu(w1 @ h1 + b1)                           # [64, L]
  out = w2 @ h2 + b2 - 2.19                         # [1, L]
32 instances (8 per image, 4 images).

Sharding (hybrid): core c -> image c//2, position half c%2. Each core runs
all 8 instances of its image over Lc = L/2 = 20480 positions, so every byte
of x is shipped to exactly one core.

Device mapping (per core, 8 instances = 4 pairs, 20 chunks of T=1024):
  - feats live in 10 resident SBUF tiles [66, 2048] bf16: rows 0-63 = x
    chunk (64-row DMA -> spreads across all 16 SDMA engines; odd partition
    counts serialize onto one engine at ~26 GB/s), row 64 = loc_x pattern,
    row 65 = (col//256) pattern (both periodic with 1024, loaded once).
    The chunk-dependent part of the loc_y term, w0y*(80*half + 4*i), is
    folded into a per-chunk L1 relu bias together with b0.
  - L1: per pair one [66,128] bf16 lhsT (2 instances side by side), two
    512-col matmuls per chunk into a [128,1024] PSUM tile.
  - L2: block-diagonal [128,128] bf16 weights per pair.
  - L3 (64->1): outputs packed across PSUM partitions: per 512-position
    group one matmul with a zero-padded [128,32] w2 slice writes a [32,512]
    window (tile_position), accumulating 16 groups per window, so one
    [128,512] PSUM bank collects 64 groups before a single bias+copy op
    and a few strided DMAs to DRAM.
  - relu work (PSUM->SBUF copies) is split greedily between ACT and DVE.
"""

import sys

if "/opt/trn_rl_repo" not in sys.path:
    sys.path.insert(0, "/opt/trn_rl_repo")

import numpy as np
import ml_dtypes

import concourse.bass as bass
import concourse.mybir as mybir
from concourse import bacc
from concourse.tile import TileContext
from concourse.bass_utils import run_bass_kernel_spmd

BF = mybir.dt.bfloat16
F32 = mybir.dt.float32
AT = mybir.ActivationFunctionType
AL = mybir.AluOpType
bf16 = ml_dtypes.bfloat16

# Problem geometry (hardcoded per spec)
N_IMG, INS_PER_IMG, C, H, W = 4, 8, 64, 160, 256
CIN = C + 2
L = H * W                       # 40960 positions per image
L1, L2_, L3_ = (C + 2) * C, C * C, C
B1OFF = L1 + L2_ + L3_          # offsets into the 8513-param vector
MASK_BIAS_SHIFT = 2.19

N_CORES = 8
NPAIR = 4                       # 8 instances per core, 2 per matmul pack
LC = L // 2                     # 20480 positions per core
T = 1024                        # positions per chunk
NCHUNK = LC // T                # 20
FT = 2048                       # positions per feats SBUF tile
NFT = LC // FT                  # 10 resident feats tiles
GROUPS = NCHUNK * NPAIR * 2     # 160 [2, 512] position-groups per core
N_BANKS = (GROUPS + 63) // 64   # 3 output PSUM bank fills (64, 64, 32)

# relu op cost estimates (ns) for greedy ACT/DVE balancing
COST_DVE = (120 + T) / 0.96
COST_ACT = (352 + T) / 1.2

_cache = {}


def _build_program():
    nc = bacc.Bacc("TRN2", target_bir_lowering=False, debug=False)

    xb = nc.dram_tensor("xb", [C, LC], BF, kind="ExternalInput")
    coords = nc.dram_tensor("coords", [2, FT], BF, kind="ExternalInput")
    w0t = [nc.dram_tensor(f"w0t{p}", [CIN, 128], BF, kind="ExternalInput")
           for p in range(NPAIR)]
    w1t = [nc.dram_tensor(f"w1t{p}", [128, 128], BF, kind="ExternalInput")
           for p in range(NPAIR)]
    w2p = [nc.dram_tensor(f"w2p{p}", [128, 512], BF, kind="ExternalInput")
           for p in range(NPAIR)]
    b0v = [nc.dram_tensor(f"b0v{p}", [128, NCHUNK], F32, kind="ExternalInput")
           for p in range(NPAIR)]
    b1v = [nc.dram_tensor(f"b1v{p}", [128, 1], F32, kind="ExternalInput")
           for p in range(NPAIR)]
    b2v = nc.dram_tensor("b2v", [128, 1], F32, kind="ExternalInput")
    # packed output: [bank, q, col]; host un-permutes (q encodes
    # chunk/pair/half/instance) — keeps each flush one big contiguous DMA.
    o = nc.dram_tensor("o", [N_BANKS * 128, 512], F32, kind="ExternalOutput")

    eng_ns = {"dve": 0.0, "act": 0.0}

    def relu(dst, src, bias_ap):
        if eng_ns["dve"] + COST_DVE <= eng_ns["act"] + COST_ACT:
            eng_ns["dve"] += COST_DVE
            if bias_ap is None:
                nc.vector.tensor_scalar(out=dst, in0=src, scalar1=0.0,
                                        scalar2=None, op0=AL.max)
            else:
                nc.vector.tensor_scalar(out=dst, in0=src, scalar1=bias_ap,
                                        scalar2=0.0, op0=AL.add, op1=AL.max)
        else:
            eng_ns["act"] += COST_ACT
            if bias_ap is None:
                nc.scalar.activation(dst, src, AT.Relu)
            else:
                nc.scalar.activation(dst, src, AT.Relu, bias=bias_ap)

    with TileContext(nc) as tc:
        with tc.tile_pool(name="consts", bufs=1) as cpool, \
             tc.tile_pool(name="zpool", bufs=3, space="PSUM") as zpool, \
             tc.tile_pool(name="opool", bufs=2, space="PSUM") as opool, \
             tc.tile_pool(name="h1pool", bufs=10) as h1pool, \
             tc.tile_pool(name="h2pool", bufs=10) as h2pool, \
             tc.tile_pool(name="spool", bufs=2) as spool:

            # weights first (small, unblock the first matmuls early)
            w0_sb, w1_sb, w2_sb, b0_sb, b1_sb = [], [], [], [], []
            for p in range(NPAIR):
                t0 = cpool.tile([CIN, 128], BF, name=f"w0sb{p}")
                nc.sync.dma_start(out=t0, in_=w0t[p].ap())
                w0_sb.append(t0)
                t1 = cpool.tile([128, 128], BF, name=f"w1sb{p}")
                nc.sync.dma_start(out=t1, in_=w1t[p].ap())
                w1_sb.append(t1)
                t2 = cpool.tile([128, 512], BF, name=f"w2sb{p}")
                nc.sync.dma_start(out=t2, in_=w2p[p].ap())
                w2_sb.append(t2)
                t3 = cpool.tile([128, NCHUNK], F32, name=f"b0sb{p}")
                nc.sync.dma_start(out=t3, in_=b0v[p].ap())
                b0_sb.append(t3)
                t4 = cpool.tile([128, 1], F32, name=f"b1sb{p}")
                nc.sync.dma_start(out=t4, in_=b1v[p].ap())
                b1_sb.append(t4)
            b2_sb = cpool.tile([128, 1], F32, name="b2sb")
            nc.sync.dma_start(out=b2_sb, in_=b2v.ap())

            # resident feats tiles: x rows via 64-partition DMAs (sync ring),
            # coord rows (periodic patterns, identical for every tile) via
            # tiny DMAs on the scalar HWDGE ring so they don't serialize
            # behind the big x loads.
            fts = []
            for j in range(NFT):
                ft = cpool.tile([CIN, FT], BF, name=f"ft{j}")
                nc.sync.dma_start(out=ft[0:C, :],
                                  in_=xb.ap()[:, j * FT:(j + 1) * FT])
                nc.scalar.dma_start(out=ft[C:CIN, :], in_=coords.ap())
                fts.append(ft)

            z1s, z2s, h1s, h2s = {}, {}, {}, {}
            obank = {"tile": None, "idx": -1}

            def flush_obank(nrows):
                ob = obank["tile"]
                b = obank["idx"]
                stage = spool.tile([128, 512], F32, name=f"stage{b}",
                                   tag="stage")
                nc.vector.tensor_scalar(out=stage[:nrows], in0=ob[:nrows],
                                        scalar1=b2_sb[:nrows, 0:1],
                                        scalar2=None, op0=AL.add)
                dst = bass.AP(o, b * 128 * 512, [[512, nrows], [1, 512]])
                nc.sync.dma_start(out=dst, in_=stage[:nrows])

            # software-pipelined emission: iter i does L1(i), L3(i-2),
            # L2(i-1); relus follow their producers.
            for i in range(NCHUNK + 2):
                # L1(i)
                if i < NCHUNK:
                    ft = fts[i // 2]
                    c0 = (i % 2) * T
                    for p in range(NPAIR):
                        z1 = zpool.tile([128, T], F32, name=f"z1_{i}_{p}",
                                        tag="z")
                        for s in range(2):
                            nc.tensor.matmul(
                                z1[:, s * 512:(s + 1) * 512], w0_sb[p],
                                ft[:, c0 + s * 512:c0 + (s + 1) * 512],
                                start=True, stop=True)
                        h1 = h1pool.tile([128, T], BF, name=f"h1_{i}_{p}",
                                         tag="h1")
                        relu(h1, z1, b0_sb[p][:, i:i + 1])
                        h1s[(i, p)] = h1

                # L3(i-2)
                j3 = i - 2
                if j3 >= 0:
                    for p in range(NPAIR):
                        h2 = h2s[(j3, p)]
                        for s in range(2):
                            g = j3 * 8 + p * 2 + s
                            lg = g % 64
                            if lg == 0:
                                obank["tile"] = opool.tile(
                                    [128, 512], F32, name=f"ob{g // 64}",
                                    tag="ob")
                                obank["idx"] = g // 64
                            jj, jv = lg // 16, lg % 16
                            nc.tensor.matmul(
                                obank["tile"][32 * jj:32 * jj + 32, :],
                                w2_sb[p][:, 32 * jv:32 * jv + 32],
                                h2[:, s * 512:(s + 1) * 512],
                                start=(jv == 0), stop=(jv == 15),
                                tile_position=(0, 32 * jj))
                            if g == GROUPS - 1:
                                flush_obank(((g % 64) + 1) * 2)
                            elif lg == 63:
                                flush_obank(128)
                        if j3 >= 1:
                            h2s.pop((j3 - 1, p), None)

                # L2(i-1)
                j2 = i - 1
                if 0 <= j2 < NCHUNK:
                    for p in range(NPAIR):
                        z2 = zpool.tile([128, T], F32, name=f"z2_{j2}_{p}",
                                        tag="z")
                        h1 = h1s.pop((j2, p))
                        for s in range(2):
                            nc.tensor.matmul(z2[:, s * 512:(s + 1) * 512],
                                             w1_sb[p],
                                             h1[:, s * 512:(s + 1) * 512],
                                             start=True, stop=True)
                        h2 = h2pool.tile([128, T], BF, name=f"h2_{j2}_{p}",
                                         tag="h2")
                        relu(h2, z2, b1_sb[p][:, 0:1])
                        h2s[(j2, p)] = h2

    nc.compile()
    return nc


def _prep_inputs(x, mask_head_params, num_ins):
    x = np.asarray(x, dtype=np.float32)
    params = np.asarray(mask_head_params, dtype=np.float32)
    num_ins = np.asarray(num_ins)
    img_idx = np.repeat(np.arange(N_IMG), num_ins)
    assert img_idx.shape[0] == N_IMG * INS_PER_IMG

    xbf = x.reshape(N_IMG, C, L).astype(bf16)

    # coord rows, periodic with T=1024 so one [2, FT] block serves all tiles:
    # row 0 = loc_x = col % 256; row 1 = (col // 256) % 4 (loc_y base).
    cols = np.arange(FT)
    coords = np.stack([cols % W, (cols // W) % 4]).astype(bf16)

    in_maps = []
    for c in range(N_CORES):
        img, half = c // 2, c % 2
        inst = [img * INS_PER_IMG + k for k in range(INS_PER_IMG)]
        m = {"xb": np.ascontiguousarray(
                 xbf[img][:, half * LC:(half + 1) * LC]),
             "coords": coords}
        for p in range(NPAIR):
            a, b = inst[2 * p], inst[2 * p + 1]
            w0_a = params[a, :L1].reshape(C, CIN)
            w0_b = params[b, :L1].reshape(C, CIN)
            # lhsT rows: [w0[:,2:].T ; w0[:,0] (loc_x) ; w0[:,1] (loc_y)]
            w0tp = np.zeros((CIN, 128), np.float32)
            for k, wv in enumerate((w0_a, w0_b)):
                colsl = slice(64 * k, 64 * k + 64)
                w0tp[0:C, colsl] = wv[:, 2:].T
                w0tp[C, colsl] = wv[:, 0]
                w0tp[C + 1, colsl] = wv[:, 1]
            m[f"w0t{p}"] = w0tp.astype(bf16)

            # per-chunk L1 bias: b0 + w0y * (80*half + 4*i)
            b0pair = np.concatenate([params[a, B1OFF:B1OFF + C],
                                     params[b, B1OFF:B1OFF + C]])
            w0y = np.concatenate([w0_a[:, 1], w0_b[:, 1]])
            ii = np.arange(NCHUNK, dtype=np.float32)
            m[f"b0v{p}"] = (b0pair[:, None] +
                            w0y[:, None] * (80.0 * half + 4.0 * ii[None, :])
                            ).astype(np.float32)

            w1_a = params[a, L1:L1 + L2_].reshape(C, C)
            w1_b = params[b, L1:L1 + L2_].reshape(C, C)
            w1tp = np.zeros((128, 128), np.float32)
            w1tp[:64, :64] = w1_a.T
            w1tp[64:, 64:] = w1_b.T
            m[f"w1t{p}"] = w1tp.astype(bf16)

            w2_a = params[a, L1 + L2_:L1 + L2_ + C]
            w2_b = params[b, L1 + L2_:L1 + L2_ + C]
            w2pair = np.zeros((128, 2), np.float32)
            w2pair[:64, 0] = w2_a
            w2pair[64:, 1] = w2_b
            w2pad = np.zeros((128, 512), np.float32)
            for j in range(16):
                w2pad[:, 32 * j + 2 * j:32 * j + 2 * j + 2] = w2pair
            m[f"w2p{p}"] = w2pad.astype(bf16)

            b1 = np.concatenate([params[a, B1OFF + C:B1OFF + 2 * C],
                                 params[b, B1OFF + C:B1OFF + 2 * C]])
            m[f"b1v{p}"] = b1.reshape(128, 1).astype(np.float32)

        # b2 per out-bank partition q = 32a + 16bb + (4p + 2s + m):
        # instance = 2p + m with p = (q%16)//4, m = q%2
        b2 = np.empty((128, 1), np.float32)
        for q in range(128):
            p = (q % 16) // 4
            mm = q % 2
            iid = inst[2 * p + mm]
            b2[q, 0] = params[iid, B1OFF + 2 * C] - MASK_BIAS_SHIFT
        m["b2v"] = b2
        in_maps.append(m)
    return in_maps


def kernel(x, mask_head_params, num_ins):
    if "nc" not in _cache:
        _cache["nc"] = _build_program()
    nc = _cache["nc"]
    in_maps = _prep_inputs(x, mask_head_params, num_ins)
    res = run_bass_kernel_spmd(nc, in_maps, core_ids=list(range(N_CORES)))
    # un-permute packed output: row b*128 + q holds (chunk 8b+2a+bb,
    # pair p, half s, inst-in-pair m) with q = 32a + 16bb + 4p + 2s + m
    q = np.arange(128)
    a, bb, cc = q // 32, (q % 32) // 16, q % 16
    p, s, m = cc // 4, (cc % 4) // 2, cc % 2
    inst_of_q = 2 * p + m
    out = np.empty((N_IMG * INS_PER_IMG, L), dtype=np.float32)
    for c in range(N_CORES):
        img, half = c // 2, c % 2
        pk = res.results[c]["o"].reshape(N_BANKS, 128, 512)
        oc = np.empty((INS_PER_IMG, LC), dtype=np.float32)
        for b in range(N_BANKS):
            chunk = 8 * b + 2 * a + bb
            valid = chunk < NCHUNK
            base = chunk * T + s * 512
            for qi in range(128):
                if valid[qi]:
                    oc[inst_of_q[qi], base[qi]:base[qi] + 512] = pk[b, qi]
        out[img * INS_PER_IMG:(img + 1) * INS_PER_IMG,
            half * LC:(half + 1) * LC] = oc
    return out.reshape(1, N_IMG * INS_PER_IMG, H, W).astype(np.float32)


# revision 24
# speedup vs baseline: 24843.3653x; 1.0465x over previous
"""CondLaneHead DynamicMaskHead kernel for 8 Trainium2 NeuronCores.

Problem: per-instance 3-layer 1x1-conv MLP over a [64,160,256] feature map.
  feats = concat([loc_x, loc_y], x[img])            # [66, L], L = 160*256
  h1 = relu(w0 @ feats + b0)                        # [64, L]
  h2 = relu(w1 @ h1 + b1)                           # [64, L]
  out = w2 @ h2 + b2 - 2.19                         # [1, L]
32 instances (8 per image, 4 images).

Sharding (hybrid): core c -> image c//2, position half c%2. Each core runs
all 8 instances of its image over Lc = L/2 = 20480 positions, so every byte
of x is shipped to exactly one core.

Device mapping (per core, 8 instances = 4 pairs, 20 chunks of T=1024):
  - feats live in 10 resident SBUF tiles [66, 2048] bf16: rows 0-63 = x
    chunk (64-row DMA -> spreads across all 16 SDMA engines; odd partition
    counts serialize onto one engine at ~26 GB/s), row 64 = loc_x pattern,
    row 65 = (col//256) pattern (both periodic with 1024, loaded once).
    The chunk-dependent part of the loc_y term, w0y*(80*half + 4*i), is
    folded into a per-chunk L1 relu bias together with b0.
  - L1: per pair one [66,128] bf16 lhsT (2 instances side by side), two
    512-col matmuls per chunk into a [128,1024] PSUM tile.
  - L2: block-diagonal [128,128] bf16 weights per pair.
  - L3 (64->1): outputs packed across PSUM partitions: per 512-position
    group one matmul with a zero-padded [128,32] w2 slice writes a [32,512]
    window (tile_position), accumulating 16 groups per window, so one
    [128,512] PSUM bank collects 64 groups before a single bias+copy op
    and a few strided DMAs to DRAM.
  - relu work (PSUM->SBUF copies) is split greedily between ACT and DVE.
"""

import sys

if "/opt/trn_rl_repo" not in sys.path:
    sys.path.insert(0, "/opt/trn_rl_repo")

import numpy as np
import ml_dtypes

import concourse.bass as bass
import concourse.mybir as mybir
from concourse import bacc
from concourse.tile import TileContext
from concourse.bass_utils import run_bass_kernel_spmd

BF = mybir.dt.bfloat16
F32 = mybir.dt.float32
AT = mybir.ActivationFunctionType
AL = mybir.AluOpType
bf16 = ml_dtypes.bfloat16

# Problem geometry (hardcoded per spec)
N_IMG, INS_PER_IMG, C, H, W = 4, 8, 64, 160, 256
CIN = C + 2
L = H * W                       # 40960 positions per image
L1, L2_, L3_ = (C + 2) * C, C * C, C
B1OFF = L1 + L2_ + L3_          # offsets into the 8513-param vector
MASK_BIAS_SHIFT = 2.19

N_CORES = 8
NPAIR = 4                       # 8 instances per core, 2 per matmul pack
LC = L // 2                     # 20480 positions per core
T = 1024                        # positions per chunk
NCHUNK = LC // T                # 20
FT = 5120                       # positions per feats SBUF tile
NFT = LC // FT                  # 4 resident feats tiles
CPF = FT // T                   # 5 chunks per feats tile
GROUPS = NCHUNK * NPAIR * 2     # 160 [2, 512] position-groups per core
N_BANKS = (GROUPS + 63) // 64   # 3 output PSUM bank fills (64, 64, 32)

# relu op cost estimates (ns) for greedy ACT/DVE balancing
COST_DVE = (120 + T) / 0.96
COST_ACT = (352 + T) / 1.2

_cache = {}


def _build_program():
    nc = bacc.Bacc("TRN2", target_bir_lowering=False, debug=False)

    # batched inputs: DMA-issue costs ~0.6us each on the SP queue, so ship
    # few big tensors. xb rows 64/65 carry the coord patterns. wbf packs
    # w1 (4x128 cols) then w2pad (4x512 cols); wf32 packs b0 (4xNCHUNK),
    # b1 (4x1), b2 (1).
    xb = nc.dram_tensor("xb", [CIN, LC], BF, kind="ExternalInput")
    w0c = nc.dram_tensor("w0c", [CIN, NPAIR * 128], BF, kind="ExternalInput")
    wbf = nc.dram_tensor("wbf", [128, NPAIR * (128 + 512)], BF,
                         kind="ExternalInput")
    wf32 = nc.dram_tensor("wf32", [128, NPAIR * (NCHUNK + 1) + 1], F32,
                          kind="ExternalInput")
    # packed output: [bank, q, col]; host un-permutes (q encodes
    # chunk/pair/half/instance) — keeps each flush one big contiguous DMA.
    o = nc.dram_tensor("o", [N_BANKS * 128, 512], F32, kind="ExternalOutput")

    eng_ns = {"dve": 0.0, "act": 0.0}

    def relu(dst, src, bias_ap):
        if eng_ns["dve"] + COST_DVE <= eng_ns["act"] + COST_ACT:
            eng_ns["dve"] += COST_DVE
            if bias_ap is None:
                nc.vector.tensor_scalar(out=dst, in0=src, scalar1=0.0,
                                        scalar2=None, op0=AL.max)
            else:
                nc.vector.tensor_scalar(out=dst, in0=src, scalar1=bias_ap,
                                        scalar2=0.0, op0=AL.add, op1=AL.max)
        else:
            eng_ns["act"] += COST_ACT
            if bias_ap is None:
                nc.scalar.activation(dst, src, AT.Relu)
            else:
                nc.scalar.activation(dst, src, AT.Relu, bias=bias_ap)

    with TileContext(nc) as tc:
        with tc.tile_pool(name="consts", bufs=1) as cpool, \
             tc.tile_pool(name="zpool", bufs=3, space="PSUM") as zpool, \
             tc.tile_pool(name="opool", bufs=2, space="PSUM") as opool, \
             tc.tile_pool(name="h1pool", bufs=10) as h1pool, \
             tc.tile_pool(name="h2pool", bufs=10) as h2pool, \
             tc.tile_pool(name="spool", bufs=2) as spool:

            # weights first (small, unblock the first matmuls early)
            w0c_sb = cpool.tile([CIN, NPAIR * 128], BF, name="w0csb")
            nc.sync.dma_start(out=w0c_sb, in_=w0c.ap())
            wbf_sb = cpool.tile([128, NPAIR * (128 + 512)], BF, name="wbfsb")
            nc.sync.dma_start(out=wbf_sb, in_=wbf.ap())
            wf32_sb = cpool.tile([128, NPAIR * (NCHUNK + 1) + 1], F32,
                                 name="wf32sb")
            nc.sync.dma_start(out=wf32_sb, in_=wf32.ap())
            w0_sb = [w0c_sb[:, p * 128:(p + 1) * 128] for p in range(NPAIR)]
            w1_sb = [wbf_sb[:, p * 128:(p + 1) * 128] for p in range(NPAIR)]
            w2_sb = [wbf_sb[:, NPAIR * 128 + p * 512:
                            NPAIR * 128 + (p + 1) * 512]
                     for p in range(NPAIR)]
            b0_sb = [wf32_sb[:, p * NCHUNK:(p + 1) * NCHUNK]
                     for p in range(NPAIR)]
            b1_sb = [wf32_sb[:, NPAIR * NCHUNK + p:NPAIR * NCHUNK + p + 1]
                     for p in range(NPAIR)]
            b2_sb = wf32_sb[:, NPAIR * (NCHUNK + 1):
                            NPAIR * (NCHUNK + 1) + 1]

            # resident feats tiles (x + coord rows in one DMA each)
            fts = []
            for j in range(NFT):
                ft = cpool.tile([CIN, FT], BF, name=f"ft{j}")
                nc.sync.dma_start(out=ft,
                                  in_=xb.ap()[:, j * FT:(j + 1) * FT])
                fts.append(ft)

            z1s, z2s, h1s, h2s = {}, {}, {}, {}
            obank = {"tile": None, "idx": -1}

            def flush_obank(nrows):
                ob = obank["tile"]
                b = obank["idx"]
                stage = spool.tile([128, 512], F32, name=f"stage{b}",
                                   tag="stage")
                nc.vector.tensor_scalar(out=stage[:nrows], in0=ob[:nrows],
                                        scalar1=b2_sb[:nrows],
                                        scalar2=None, op0=AL.add)
                dst = bass.AP(o, b * 128 * 512, [[512, nrows], [1, 512]])
                nc.sync.dma_start(out=dst, in_=stage[:nrows])

            # software-pipelined emission: iter i does L1(i), L3(i-2),
            # L2(i-1); relus follow their producers.
            for i in range(NCHUNK + 2):
                # L1(i)
                if i < NCHUNK:
                    ft = fts[i // CPF]
                    c0 = (i % CPF) * T
                    for p in range(NPAIR):
                        z1 = zpool.tile([128, T], F32, name=f"z1_{i}_{p}",
                                        tag="z")
                        for s in range(2):
                            nc.tensor.matmul(
                                z1[:, s * 512:(s + 1) * 512], w0_sb[p],
                                ft[:, c0 + s * 512:c0 + (s + 1) * 512],
                                start=True, stop=True)
                        h1 = h1pool.tile([128, T], BF, name=f"h1_{i}_{p}",
                                         tag="h1")
                        relu(h1, z1, b0_sb[p][:, i:i + 1])
                        h1s[(i, p)] = h1

                # L3(i-2)
                j3 = i - 2
                if j3 >= 0:
                    for p in range(NPAIR):
                        h2 = h2s[(j3, p)]
                        for s in range(2):
                            g = j3 * 8 + p * 2 + s
                            lg = g % 64
                            if lg == 0:
                                obank["tile"] = opool.tile(
                                    [128, 512], F32, name=f"ob{g // 64}",
                                    tag="ob")
                                obank["idx"] = g // 64
                            jj, jv = lg // 16, lg % 16
                            nc.tensor.matmul(
                                obank["tile"][32 * jj:32 * jj + 32, :],
                                w2_sb[p][:, 32 * jv:32 * jv + 32],
                                h2[:, s * 512:(s + 1) * 512],
                                start=(jv == 0), stop=(jv == 15),
                                tile_position=(0, 32 * jj))
                            if g == GROUPS - 1:
                                flush_obank(((g % 64) + 1) * 2)
                            elif lg == 63:
                                flush_obank(128)
                        if j3 >= 1:
                            h2s.pop((j3 - 1, p), None)

                # L2(i-1)
                j2 = i - 1
                if 0 <= j2 < NCHUNK:
                    for p in range(NPAIR):
                        z2 = zpool.tile([128, T], F32, name=f"z2_{j2}_{p}",
                                        tag="z")
                        h1 = h1s.pop((j2, p))
                        for s in range(2):
                            nc.tensor.matmul(z2[:, s * 512:(s + 1) * 512],
                                             w1_sb[p],
                                             h1[:, s * 512:(s + 1) * 512],
                                             start=True, stop=True)
                        h2 = h2pool.tile([128, T], BF, name=f"h2_{j2}_{p}",
                                         tag="h2")
                        relu(h2, z2, b1_sb[p])
                        h2s[(j2, p)] = h2

    nc.compile()
    return nc


def _prep_inputs(x, mask_head_params, num_ins):
    x = np.asarray(x, dtype=np.float32)
    params = np.asarray(mask_head_params, dtype=np.float32)
    num_ins = np.asarray(num_ins)
    img_idx = np.repeat(np.arange(N_IMG), num_ins)
    assert img_idx.shape[0] == N_IMG * INS_PER_IMG

    xbf = x.reshape(N_IMG, C, L).astype(bf16)

    # coord rows, periodic with T=1024: loc_x = col % 256 and the loc_y
    # in-chunk base (col // 256) % 4; chunk offsets fold into the L1 bias.
    cols = np.arange(LC)
    coords = np.stack([cols % W, (cols // W) % 4]).astype(bf16)

    in_maps = []
    for c in range(N_CORES):
        img, half = c // 2, c % 2
        inst = [img * INS_PER_IMG + k for k in range(INS_PER_IMG)]
        xbc = np.empty((CIN, LC), dtype=bf16)
        xbc[0:C] = xbf[img][:, half * LC:(half + 1) * LC]
        xbc[C:CIN] = coords
        m = {"xb": xbc}
        w0cat = np.zeros((CIN, NPAIR * 128), np.float32)
        wbf = np.zeros((128, NPAIR * (128 + 512)), np.float32)
        wf32 = np.zeros((128, NPAIR * (NCHUNK + 1) + 1), np.float32)
        for p in range(NPAIR):
            a, b = inst[2 * p], inst[2 * p + 1]
            w0_a = params[a, :L1].reshape(C, CIN)
            w0_b = params[b, :L1].reshape(C, CIN)
            # lhsT rows: [w0[:,2:].T ; w0[:,0] (loc_x) ; w0[:,1] (loc_y)]
            for k, wv in enumerate((w0_a, w0_b)):
                colsl = slice(p * 128 + 64 * k, p * 128 + 64 * k + 64)
                w0cat[0:C, colsl] = wv[:, 2:].T
                w0cat[C, colsl] = wv[:, 0]
                w0cat[C + 1, colsl] = wv[:, 1]

            # per-chunk L1 bias: b0 + w0y * (80*half + 4*i)
            b0pair = np.concatenate([params[a, B1OFF:B1OFF + C],
                                     params[b, B1OFF:B1OFF + C]])
            w0y = np.concatenate([w0_a[:, 1], w0_b[:, 1]])
            ii = np.arange(NCHUNK, dtype=np.float32)
            wf32[:, p * NCHUNK:(p + 1) * NCHUNK] = (
                b0pair[:, None] +
                w0y[:, None] * (80.0 * half + 4.0 * ii[None, :]))

            w1_a = params[a, L1:L1 + L2_].reshape(C, C)
            w1_b = params[b, L1:L1 + L2_].reshape(C, C)
            wbf[:64, p * 128:p * 128 + 64] = w1_a.T
            wbf[64:, p * 128 + 64:(p + 1) * 128] = w1_b.T

            w2_a = params[a, L1 + L2_:L1 + L2_ + C]
            w2_b = params[b, L1 + L2_:L1 + L2_ + C]
            w2pair = np.zeros((128, 2), np.float32)
            w2pair[:64, 0] = w2_a
            w2pair[64:, 1] = w2_b
            base = NPAIR * 128 + p * 512
            for j in range(16):
                wbf[:, base + 34 * j:base + 34 * j + 2] = w2pair

            wf32[:, NPAIR * NCHUNK + p] = np.concatenate(
                [params[a, B1OFF + C:B1OFF + 2 * C],
                 params[b, B1OFF + C:B1OFF + 2 * C]])

        # b2 per out-bank partition q = 32a + 16bb + (4p + 2s + m):
        # instance = 2p + m with p = (q%16)//4, m = q%2
        for q in range(128):
            p = (q % 16) // 4
            mm = q % 2
            iid = inst[2 * p + mm]
            wf32[q, NPAIR * (NCHUNK + 1)] = (params[iid, B1OFF + 2 * C]
                                             - MASK_BIAS_SHIFT)
        m["w0c"] = w0cat.astype(bf16)
        m["wbf"] = wbf.astype(bf16)
        m["wf32"] = wf32
        in_maps.append(m)
    return in_maps


def kernel(x, mask_head_params, num_ins):
    if "nc" not in _cache:
        _cache["nc"] = _build_program()
    nc = _cache["nc"]
    in_maps = _prep_inputs(x, mask_head_params, num_ins)
    res = run_bass_kernel_spmd(nc, in_maps, core_ids=list(range(N_CORES)))
    # un-permute packed output: row b*128 + q holds (chunk 8b+2a+bb,
    # pair p, half s, inst-in-pair m) with q = 32a + 16bb + 4p + 2s + m
    q = np.arange(128)
    a, bb, cc = q // 32, (q % 32) // 16, q % 16
    p, s, m = cc // 4, (cc % 4) // 2, cc % 2
    inst_of_q = 2 * p + m
    out = np.empty((N_IMG * INS_PER_IMG, L), dtype=np.float32)
    for c in range(N_CORES):
        img, half = c // 2, c % 2
        pk = res.results[c]["o"].reshape(N_BANKS, 128, 512)
        oc = np.empty((INS_PER_IMG, LC), dtype=np.float32)
        for b in range(N_BANKS):
            chunk = 8 * b + 2 * a + bb
            valid = chunk < NCHUNK
            base = chunk * T + s * 512
            for qi in range(128):
                if valid[qi]:
                    oc[inst_of_q[qi], base[qi]:base[qi] + 512] = pk[b, qi]
        out[img * INS_PER_IMG:(img + 1) * INS_PER_IMG,
            half * LC:(half + 1) * LC] = oc
    return out.reshape(1, N_IMG * INS_PER_IMG, H, W).astype(np.float32)


# revision 27
# speedup vs baseline: 26120.3925x; 1.0514x over previous
"""CondLaneHead DynamicMaskHead kernel for 8 Trainium2 NeuronCores.

Problem: per-instance 3-layer 1x1-conv MLP over a [64,160,256] feature map.
  feats = concat([loc_x, loc_y], x[img])            # [66, L], L = 160*256
  h1 = relu(w0 @ feats + b0)                        # [64, L]
  h2 = relu(w1 @ h1 + b1)                           # [64, L]
  out = w2 @ h2 + b2 - 2.19                         # [1, L]
32 instances (8 per image, 4 images).

Sharding (hybrid): core c -> image c//2, position half c%2. Each core runs
all 8 instances of its image over Lc = L/2 = 20480 positions, so every byte
of x is shipped to exactly one core.

Device mapping (per core, 8 instances = 4 pairs, 20 chunks of T=1024):
  - feats live in 10 resident SBUF tiles [66, 2048] bf16: rows 0-63 = x
    chunk (64-row DMA -> spreads across all 16 SDMA engines; odd partition
    counts serialize onto one engine at ~26 GB/s), row 64 = loc_x pattern,
    row 65 = (col//256) pattern (both periodic with 1024, loaded once).
    The chunk-dependent part of the loc_y term, w0y*(80*half + 4*i), is
    folded into a per-chunk L1 relu bias together with b0.
  - L1: per pair one [66,128] bf16 lhsT (2 instances side by side), two
    512-col matmuls per chunk into a [128,1024] PSUM tile.
  - L2: block-diagonal [128,128] bf16 weights per pair.
  - L3 (64->1): outputs packed across PSUM partitions: per 512-position
    group one matmul with a zero-padded [128,32] w2 slice writes a [32,512]
    window (tile_position), accumulating 16 groups per window, so one
    [128,512] PSUM bank collects 64 groups before a single bias+copy op
    and a few strided DMAs to DRAM.
  - relu work (PSUM->SBUF copies) is split greedily between ACT and DVE.
"""

import sys

if "/opt/trn_rl_repo" not in sys.path:
    sys.path.insert(0, "/opt/trn_rl_repo")

import numpy as np
import ml_dtypes

import concourse.bass as bass
import concourse.mybir as mybir
from concourse import bacc
from concourse.tile import TileContext
from concourse.bass_utils import run_bass_kernel_spmd

BF = mybir.dt.bfloat16
F32 = mybir.dt.float32
AT = mybir.ActivationFunctionType
AL = mybir.AluOpType
bf16 = ml_dtypes.bfloat16

# Problem geometry (hardcoded per spec)
N_IMG, INS_PER_IMG, C, H, W = 4, 8, 64, 160, 256
CIN = C + 2
L = H * W                       # 40960 positions per image
L1, L2_, L3_ = (C + 2) * C, C * C, C
B1OFF = L1 + L2_ + L3_          # offsets into the 8513-param vector
MASK_BIAS_SHIFT = 2.19

N_CORES = 8
NPAIR = 4                       # 8 instances per core, 2 per matmul pack
LC = L // 2                     # 20480 positions per core
T = 1024                        # positions per chunk
NCHUNK = LC // T                # 20
FT = 5120                       # positions per feats SBUF tile
NFT = LC // FT                  # 4 resident feats tiles
CPF = FT // T                   # 5 chunks per feats tile
GROUPS = NCHUNK * NPAIR * 2     # 160 [2, 512] position-groups per core
N_BANKS = (GROUPS + 63) // 64   # 3 output PSUM bank fills (64, 64, 32)

# relu op cost estimates (ns) for greedy ACT/DVE balancing
COST_DVE = (120 + T) / 0.96
COST_ACT = (352 + T) / 1.2

_cache = {}


def _build_program():
    nc = bacc.Bacc("TRN2", target_bir_lowering=False, debug=False)

    # batched inputs: DMA-issue costs ~0.6us each on the SP queue, so ship
    # few big tensors. xb rows 64/65 carry the coord patterns. wbf packs
    # w1 (4x128 cols) then w2pad (4x512 cols); wf32 packs b0 (4xNCHUNK),
    # b1 (4x1), b2 (1).
    xb = nc.dram_tensor("xb", [CIN, LC], BF, kind="ExternalInput")
    w0c = nc.dram_tensor("w0c", [CIN, NPAIR * 128], BF, kind="ExternalInput")
    wbf = nc.dram_tensor("wbf", [128, NPAIR * (128 + 512)], BF,
                         kind="ExternalInput")
    wf32 = nc.dram_tensor("wf32", [128, NPAIR * (NCHUNK + 1) + 1], F32,
                          kind="ExternalInput")
    # packed output: [bank, q, col]; host un-permutes (q encodes
    # chunk/pair/half/instance) — keeps each flush one big contiguous DMA.
    o = nc.dram_tensor("o", [N_BANKS * 128, 512], F32, kind="ExternalOutput")

    eng_ns = {"dve": 0.0, "act": 0.0}

    def relu(dst, src, bias_ap):
        if eng_ns["dve"] + COST_DVE <= eng_ns["act"] + COST_ACT:
            eng_ns["dve"] += COST_DVE
            if bias_ap is None:
                nc.vector.tensor_scalar(out=dst, in0=src, scalar1=0.0,
                                        scalar2=None, op0=AL.max)
            else:
                nc.vector.tensor_scalar(out=dst, in0=src, scalar1=bias_ap,
                                        scalar2=0.0, op0=AL.add, op1=AL.max)
        else:
            eng_ns["act"] += COST_ACT
            if bias_ap is None:
                nc.scalar.activation(dst, src, AT.Relu)
            else:
                nc.scalar.activation(dst, src, AT.Relu, bias=bias_ap)

    with TileContext(nc) as tc:
        with tc.tile_pool(name="consts", bufs=1) as cpool, \
             tc.tile_pool(name="zpool", bufs=3, space="PSUM") as zpool, \
             tc.tile_pool(name="opool", bufs=2, space="PSUM") as opool, \
             tc.tile_pool(name="h1pool", bufs=10) as h1pool, \
             tc.tile_pool(name="h2pool", bufs=16) as h2pool, \
             tc.tile_pool(name="spool", bufs=2) as spool:

            # DMA order: what the first chunk needs first (w0c + ft0), then
            # relu bias (wf32), then L2/L3 weights, then remaining feats.
            w0c_sb = cpool.tile([CIN, NPAIR * 128], BF, name="w0csb")
            nc.sync.dma_start(out=w0c_sb, in_=w0c.ap())
            ft0 = cpool.tile([CIN, FT], BF, name="ft0")
            nc.sync.dma_start(out=ft0, in_=xb.ap()[:, 0:FT])
            wf32_sb = cpool.tile([128, NPAIR * (NCHUNK + 1) + 1], F32,
                                 name="wf32sb")
            nc.sync.dma_start(out=wf32_sb, in_=wf32.ap())
            wbf_sb = cpool.tile([128, NPAIR * (128 + 512)], BF, name="wbfsb")
            nc.sync.dma_start(out=wbf_sb, in_=wbf.ap())
            w0_sb = [w0c_sb[:, p * 128:(p + 1) * 128] for p in range(NPAIR)]
            w1_sb = [wbf_sb[:, p * 128:(p + 1) * 128] for p in range(NPAIR)]
            w2_sb = [wbf_sb[:, NPAIR * 128 + p * 512:
                            NPAIR * 128 + (p + 1) * 512]
                     for p in range(NPAIR)]
            b0_sb = [wf32_sb[:, p * NCHUNK:(p + 1) * NCHUNK]
                     for p in range(NPAIR)]
            b1_sb = [wf32_sb[:, NPAIR * NCHUNK + p:NPAIR * NCHUNK + p + 1]
                     for p in range(NPAIR)]
            b2_sb = wf32_sb[:, NPAIR * (NCHUNK + 1):
                            NPAIR * (NCHUNK + 1) + 1]

            # remaining resident feats tiles (x + coord rows in one DMA each)
            fts = [ft0]
            for j in range(1, NFT):
                ft = cpool.tile([CIN, FT], BF, name=f"ft{j}")
                nc.sync.dma_start(out=ft,
                                  in_=xb.ap()[:, j * FT:(j + 1) * FT])
                fts.append(ft)

            z1s, z2s, h1s, h2s = {}, {}, {}, {}
            obank = {"tile": None, "idx": -1}

            def flush_obank(nrows):
                ob = obank["tile"]
                b = obank["idx"]
                stage = spool.tile([128, 512], F32, name=f"stage{b}",
                                   tag="stage")
                nc.vector.tensor_scalar(out=stage[:nrows], in0=ob[:nrows],
                                        scalar1=b2_sb[:nrows],
                                        scalar2=None, op0=AL.add)
                dst = bass.AP(o, b * 128 * 512, [[512, nrows], [1, 512]])
                nc.sync.dma_start(out=dst, in_=stage[:nrows])

            # software-pipelined emission: iter i does L1(i), L3(i-2),
            # L2(i-1); relus follow their producers.
            for i in range(NCHUNK + 3):
                # L1(i)
                if i < NCHUNK:
                    ft = fts[i // CPF]
                    c0 = (i % CPF) * T
                    for p in range(NPAIR):
                        z1 = zpool.tile([128, T], F32, name=f"z1_{i}_{p}",
                                        tag="z")
                        for s in range(2):
                            nc.tensor.matmul(
                                z1[:, s * 512:(s + 1) * 512], w0_sb[p],
                                ft[:, c0 + s * 512:c0 + (s + 1) * 512],
                                start=True, stop=True)
                        h1 = h1pool.tile([128, T], BF, name=f"h1_{i}_{p}",
                                         tag="h1")
                        relu(h1, z1, b0_sb[p][:, i:i + 1])
                        h1s[(i, p)] = h1

                # L3(i-3)
                j3 = i - 3
                if j3 >= 0:
                    for p in range(NPAIR):
                        h2 = h2s[(j3, p)]
                        for s in range(2):
                            g = j3 * 8 + p * 2 + s
                            lg = g % 64
                            if lg == 0:
                                obank["tile"] = opool.tile(
                                    [128, 512], F32, name=f"ob{g // 64}",
                                    tag="ob")
                                obank["idx"] = g // 64
                            jj, jv = lg // 16, lg % 16
                            nc.tensor.matmul(
                                obank["tile"][32 * jj:32 * jj + 32, :],
                                w2_sb[p][:, 32 * jv:32 * jv + 32],
                                h2[:, s * 512:(s + 1) * 512],
                                start=(jv == 0), stop=(jv == 15),
                                tile_position=(0, 32 * jj))
                            if g == GROUPS - 1:
                                flush_obank(((g % 64) + 1) * 2)
                            elif lg == 63:
                                flush_obank(128)
                        if j3 >= 1:
                            h2s.pop((j3 - 1, p), None)

                # L2(i-1)
                j2 = i - 1
                if 0 <= j2 < NCHUNK:
                    for p in range(NPAIR):
                        z2 = zpool.tile([128, T], F32, name=f"z2_{j2}_{p}",
                                        tag="z")
                        h1 = h1s.pop((j2, p))
                        for s in range(2):
                            nc.tensor.matmul(z2[:, s * 512:(s + 1) * 512],
                                             w1_sb[p],
                                             h1[:, s * 512:(s + 1) * 512],
                                             start=True, stop=True)
                        h2 = h2pool.tile([128, T], BF, name=f"h2_{j2}_{p}",
                                         tag="h2")
                        relu(h2, z2, b1_sb[p])
                        h2s[(j2, p)] = h2

    nc.compile()
    return nc


def _prep_inputs(x, mask_head_params, num_ins):
    x = np.asarray(x, dtype=np.float32)
    params = np.asarray(mask_head_params, dtype=np.float32)
    num_ins = np.asarray(num_ins)
    img_idx = np.repeat(np.arange(N_IMG), num_ins)
    assert img_idx.shape[0] == N_IMG * INS_PER_IMG

    xbf = x.reshape(N_IMG, C, L).astype(bf16)

    # coord rows, periodic with T=1024: loc_x = col % 256 and the loc_y
    # in-chunk base (col // 256) % 4; chunk offsets fold into the L1 bias.
    cols = np.arange(LC)
    coords = np.stack([cols % W, (cols // W) % 4]).astype(bf16)

    in_maps = []
    for c in range(N_CORES):
        img, half = c // 2, c % 2
        inst = [img * INS_PER_IMG + k for k in range(INS_PER_IMG)]
        xbc = np.empty((CIN, LC), dtype=bf16)
        xbc[0:C] = xbf[img][:, half * LC:(half + 1) * LC]
        xbc[C:CIN] = coords
        m = {"xb": xbc}
        w0cat = np.zeros((CIN, NPAIR * 128), np.float32)
        wbf = np.zeros((128, NPAIR * (128 + 512)), np.float32)
        wf32 = np.zeros((128, NPAIR * (NCHUNK + 1) + 1), np.float32)
        for p in range(NPAIR):
            a, b = inst[2 * p], inst[2 * p + 1]
            w0_a = params[a, :L1].reshape(C, CIN)
            w0_b = params[b, :L1].reshape(C, CIN)
            # lhsT rows: [w0[:,2:].T ; w0[:,0] (loc_x) ; w0[:,1] (loc_y)]
            for k, wv in enumerate((w0_a, w0_b)):
                colsl = slice(p * 128 + 64 * k, p * 128 + 64 * k + 64)
                w0cat[0:C, colsl] = wv[:, 2:].T
                w0cat[C, colsl] = wv[:, 0]
                w0cat[C + 1, colsl] = wv[:, 1]

            # per-chunk L1 bias: b0 + w0y * (80*half + 4*i)
            b0pair = np.concatenate([params[a, B1OFF:B1OFF + C],
                                     params[b, B1OFF:B1OFF + C]])
            w0y = np.concatenate([w0_a[:, 1], w0_b[:, 1]])
            ii = np.arange(NCHUNK, dtype=np.float32)
            wf32[:, p * NCHUNK:(p + 1) * NCHUNK] = (
                b0pair[:, None] +
                w0y[:, None] * (80.0 * half + 4.0 * ii[None, :]))

            w1_a = params[a, L1:L1 + L2_].reshape(C, C)
            w1_b = params[b, L1:L1 + L2_].reshape(C, C)
            wbf[:64, p * 128:p * 128 + 64] = w1_a.T
            wbf[64:, p * 128 + 64:(p + 1) * 128] = w1_b.T

            w2_a = params[a, L1 + L2_:L1 + L2_ + C]
            w2_b = params[b, L1 + L2_:L1 + L2_ + C]
            w2pair = np.zeros((128, 2), np.float32)
            w2pair[:64, 0] = w2_a
            w2pair[64:, 1] = w2_b
            base = NPAIR * 128 + p * 512
            for j in range(16):
                wbf[:, base + 34 * j:base + 34 * j + 2] = w2pair

            wf32[:, NPAIR * NCHUNK + p] = np.concatenate(
                [params[a, B1OFF + C:B1OFF + 2 * C],
                 params[b, B1OFF + C:B1OFF + 2 * C]])

        # b2 per out-bank partition q = 32a + 16bb + (4p + 2s + m):
        # instance = 2p + m with p = (q%16)//4, m = q%2
        for q in range(128):
            p = (q % 16) // 4
            mm = q % 2
            iid = inst[2 * p + mm]
            wf32[q, NPAIR * (NCHUNK + 1)] = (params[iid, B1OFF + 2 * C]
                                             - MASK_BIAS_SHIFT)
        m["w0c"] = w0cat.astype(bf16)
        m["wbf"] = wbf.astype(bf16)
        m["wf32"] = wf32
        in_maps.append(m)
    return in_maps


def kernel(x, mask_head_params, num_ins):
    if "nc" not in _cache:
        _cache["nc"] = _build_program()
    nc = _cache["nc"]
    in_maps = _prep_inputs(x, mask_head_params, num_ins)
    res = run_bass_kernel_spmd(nc, in_maps, core_ids=list(range(N_CORES)))
    # un-permute packed output: row b*128 + q holds (chunk 8b+2a+bb,
    # pair p, half s, inst-in-pair m) with q = 32a + 16bb + 4p + 2s + m
    q = np.arange(128)
    a, bb, cc = q // 32, (q % 32) // 16, q % 16
    p, s, m = cc // 4, (cc % 4) // 2, cc % 2
    inst_of_q = 2 * p + m
    out = np.empty((N_IMG * INS_PER_IMG, L), dtype=np.float32)
    for c in range(N_CORES):
        img, half = c // 2, c % 2
        pk = res.results[c]["o"].reshape(N_BANKS, 128, 512)
        oc = np.empty((INS_PER_IMG, LC), dtype=np.float32)
        for b in range(N_BANKS):
            chunk = 8 * b + 2 * a + bb
            valid = chunk < NCHUNK
            base = chunk * T + s * 512
            for qi in range(128):
                if valid[qi]:
                    oc[inst_of_q[qi], base[qi]:base[qi] + 512] = pk[b, qi]
        out[img * INS_PER_IMG:(img + 1) * INS_PER_IMG,
            half * LC:(half + 1) * LC] = oc
    return out.reshape(1, N_IMG * INS_PER_IMG, H, W).astype(np.float32)


# revision 33
# speedup vs baseline: 26693.6774x; 1.0219x over previous
"""CondLaneHead DynamicMaskHead kernel for 8 Trainium2 NeuronCores.

Problem: per-instance 3-layer 1x1-conv MLP over a [64,160,256] feature map.
  feats = concat([loc_x, loc_y], x[img])            # [66, L], L = 160*256
  h1 = relu(w0 @ feats + b0)                        # [64, L]
  h2 = relu(w1 @ h1 + b1)                           # [64, L]
  out = w2 @ h2 + b2 - 2.19                         # [1, L]
32 instances (8 per image, 4 images).

Sharding (hybrid): core c -> image c//2, position half c%2. Each core runs
all 8 instances of its image over Lc = L/2 = 20480 positions, so every byte
of x is shipped to exactly one core.

Device mapping (per core, 8 instances = 4 pairs, 20 chunks of T=1024):
  - feats live in 10 resident SBUF tiles [66, 2048] bf16: rows 0-63 = x
    chunk (64-row DMA -> spreads across all 16 SDMA engines; odd partition
    counts serialize onto one engine at ~26 GB/s), row 64 = loc_x pattern,
    row 65 = (col//256) pattern (both periodic with 1024, loaded once).
    The chunk-dependent part of the loc_y term, w0y*(80*half + 4*i), is
    folded into a per-chunk L1 relu bias together with b0.
  - L1: per pair one [66,128] bf16 lhsT (2 instances side by side), two
    512-col matmuls per chunk into a [128,1024] PSUM tile.
  - L2: block-diagonal [128,128] bf16 weights per pair.
  - L3 (64->1): outputs packed across PSUM partitions: per 512-position
    group one matmul with a zero-padded [128,32] w2 slice writes a [32,512]
    window (tile_position), accumulating 16 groups per window, so one
    [128,512] PSUM bank collects 64 groups before a single bias+copy op
    and a few strided DMAs to DRAM.
  - relu work (PSUM->SBUF copies) is split greedily between ACT and DVE.
"""

import sys

if "/opt/trn_rl_repo" not in sys.path:
    sys.path.insert(0, "/opt/trn_rl_repo")

import numpy as np
import ml_dtypes

import concourse.bass as bass
import concourse.mybir as mybir
from concourse import bacc
from concourse.tile import TileContext
from concourse.bass_utils import run_bass_kernel_spmd

BF = mybir.dt.bfloat16
F32 = mybir.dt.float32
AT = mybir.ActivationFunctionType
AL = mybir.AluOpType
bf16 = ml_dtypes.bfloat16

# Problem geometry (hardcoded per spec)
N_IMG, INS_PER_IMG, C, H, W = 4, 8, 64, 160, 256
CIN = C + 2
L = H * W                       # 40960 positions per image
L1, L2_, L3_ = (C + 2) * C, C * C, C
B1OFF = L1 + L2_ + L3_          # offsets into the 8513-param vector
MASK_BIAS_SHIFT = 2.19

N_CORES = 8
NPAIR = 4                       # 8 instances per core, 2 per matmul pack
LC = L // 2                     # 20480 positions per core
T = 1024                        # positions per chunk
NCHUNK = LC // T                # 20
FT = 5120                       # positions per feats SBUF tile
NFT = LC // FT                  # 4 resident feats tiles
CPF = FT // T                   # 5 chunks per feats tile
GROUPS = NCHUNK * NPAIR * 2     # 160 [2, 512] position-groups per core
N_BANKS = (GROUPS + 63) // 64   # 3 output PSUM bank fills (64, 64, 32)

# relu op cost estimates (ns, HW-measured) for greedy ACT/DVE balancing
COST_DVE = 1258.0
COST_ACT = 1165.0

_cache = {}


def _build_program():
    nc = bacc.Bacc("TRN2", target_bir_lowering=False, debug=False)

    # batched inputs: DMA-issue costs ~0.6us each on the SP queue, so ship
    # few big tensors. xb rows 64/65 carry the coord patterns. wbf packs
    # w1 (4x128 cols) then w2pad (4x512 cols); wf32 packs b0 (4xNCHUNK),
    # b1 (4x1), b2 (1).
    xb = nc.dram_tensor("xb", [CIN, LC], BF, kind="ExternalInput")
    w0c = nc.dram_tensor("w0c", [CIN, NPAIR * 128], BF, kind="ExternalInput")
    wbf = nc.dram_tensor("wbf", [128, NPAIR * (128 + 512)], BF,
                         kind="ExternalInput")
    wf32 = nc.dram_tensor("wf32", [128, NPAIR * (NCHUNK + 1) + 1], F32,
                          kind="ExternalInput")
    # packed output: [bank, q, col]; host un-permutes (q encodes
    # chunk/pair/half/instance) — keeps each flush one big contiguous DMA.
    o = nc.dram_tensor("o", [N_BANKS * 128, 512], F32, kind="ExternalOutput")

    eng_ns = {"dve": 0.0, "act": 0.0}

    def relu(dst, src, bias_ap):
        if eng_ns["dve"] + COST_DVE <= eng_ns["act"] + COST_ACT:
            eng_ns["dve"] += COST_DVE
            if bias_ap is None:
                nc.vector.tensor_scalar(out=dst, in0=src, scalar1=0.0,
                                        scalar2=None, op0=AL.max)
            else:
                nc.vector.tensor_scalar(out=dst, in0=src, scalar1=bias_ap,
                                        scalar2=0.0, op0=AL.add, op1=AL.max)
        else:
            eng_ns["act"] += COST_ACT
            if bias_ap is None:
                nc.scalar.activation(dst, src, AT.Relu)
            else:
                nc.scalar.activation(dst, src, AT.Relu, bias=bias_ap)

    with TileContext(nc) as tc:
        with tc.tile_pool(name="consts", bufs=1) as cpool, \
             tc.tile_pool(name="zpool", bufs=3, space="PSUM") as zpool, \
             tc.tile_pool(name="opool", bufs=2, space="PSUM") as opool, \
             tc.tile_pool(name="h1pool", bufs=10) as h1pool, \
             tc.tile_pool(name="h2pool", bufs=16) as h2pool, \
             tc.tile_pool(name="spool", bufs=2) as spool:

            # DMA order: what the first chunk needs first (w0c + a small
            # first feats piece), then relu bias (wf32), then L2/L3 weights,
            # then the rest of the feats.
            w0c_sb = cpool.tile([CIN, NPAIR * 128], BF, name="w0csb")
            nc.sync.dma_start(out=w0c_sb, in_=w0c.ap())
            ft0a = cpool.tile([CIN, T], BF, name="ft0a")
            nc.sync.dma_start(out=ft0a, in_=xb.ap()[:, 0:T])
            wf32_sb = cpool.tile([128, NPAIR * (NCHUNK + 1) + 1], F32,
                                 name="wf32sb")
            nc.sync.dma_start(out=wf32_sb, in_=wf32.ap())
            ft0b = cpool.tile([CIN, FT - T], BF, name="ft0b")
            nc.sync.dma_start(out=ft0b, in_=xb.ap()[:, T:FT])
            wbf_sb = cpool.tile([128, NPAIR * (128 + 512)], BF, name="wbfsb")
            nc.sync.dma_start(out=wbf_sb, in_=wbf.ap())
            w0_sb = [w0c_sb[:, p * 128:(p + 1) * 128] for p in range(NPAIR)]
            w1_sb = [wbf_sb[:, p * 128:(p + 1) * 128] for p in range(NPAIR)]
            w2_sb = [wbf_sb[:, NPAIR * 128 + p * 512:
                            NPAIR * 128 + (p + 1) * 512]
                     for p in range(NPAIR)]
            b0_sb = [wf32_sb[:, p * NCHUNK:(p + 1) * NCHUNK]
                     for p in range(NPAIR)]
            b1_sb = [wf32_sb[:, NPAIR * NCHUNK + p:NPAIR * NCHUNK + p + 1]
                     for p in range(NPAIR)]
            b2_sb = wf32_sb[:, NPAIR * (NCHUNK + 1):
                            NPAIR * (NCHUNK + 1) + 1]

            # remaining resident feats tiles (x + coord rows in one DMA each)
            fts = [None]
            for j in range(1, NFT):
                ft = cpool.tile([CIN, FT], BF, name=f"ft{j}")
                nc.sync.dma_start(out=ft,
                                  in_=xb.ap()[:, j * FT:(j + 1) * FT])
                fts.append(ft)

            def feat_slice(i, s):
                c = i * T + s * 512
                if c < T:
                    return ft0a[:, c:c + 512]
                if c < FT:
                    return ft0b[:, c - T:c - T + 512]
                return fts[i // CPF][:, c - (i // CPF) * FT:
                                     c - (i // CPF) * FT + 512]

            z1s, z2s, h1s, h2s = {}, {}, {}, {}
            obank = {"tile": None, "idx": -1}

            def flush_obank(nrows):
                ob = obank["tile"]
                b = obank["idx"]
                stage = spool.tile([128, 512], F32, name=f"stage{b}",
                                   tag="stage")
                nc.scalar.activation(stage[:nrows], ob[:nrows], AT.Identity,
                                     bias=b2_sb[:nrows])
                eng_ns["act"] += 750.0
                dst = bass.AP(o, b * 128 * 512, [[512, nrows], [1, 512]])
                nc.sync.dma_start(out=dst, in_=stage[:nrows])

            # software-pipelined emission: iter i does L1(i), L3(i-2),
            # L2(i-1); relus follow their producers.
            for i in range(NCHUNK + 3):
                # L1(i)
                if i < NCHUNK:
                    for p in range(NPAIR):
                        z1 = zpool.tile([128, T], F32, name=f"z1_{i}_{p}",
                                        tag="z")
                        for s in range(2):
                            nc.tensor.matmul(
                                z1[:, s * 512:(s + 1) * 512], w0_sb[p],
                                feat_slice(i, s),
                                start=True, stop=True)
                        h1 = h1pool.tile([128, T], BF, name=f"h1_{i}_{p}",
                                         tag="h1")
                        relu(h1, z1, b0_sb[p][:, i:i + 1])
                        h1s[(i, p)] = h1

                # L3(i-3)
                j3 = i - 3
                if j3 >= 0:
                    for p in range(NPAIR):
                        h2 = h2s[(j3, p)]
                        for s in range(2):
                            g = j3 * 8 + p * 2 + s
                            lg = g % 64
                            if lg == 0:
                                obank["tile"] = opool.tile(
                                    [128, 512], F32, name=f"ob{g // 64}",
                                    tag="ob")
                                obank["idx"] = g // 64
                            jj, jv = lg // 16, lg % 16
                            nc.tensor.matmul(
                                obank["tile"][32 * jj:32 * jj + 32, :],
                                w2_sb[p][:, 32 * jv:32 * jv + 32],
                                h2[:, s * 512:(s + 1) * 512],
                                start=(jv == 0), stop=(jv == 15),
                                tile_position=(0, 32 * jj))
                            if g == GROUPS - 1:
                                flush_obank(((g % 64) + 1) * 2)
                            elif lg == 63:
                                flush_obank(128)
                        if j3 >= 1:
                            h2s.pop((j3 - 1, p), None)

                # L2(i-1)
                j2 = i - 1
                if 0 <= j2 < NCHUNK:
                    for p in range(NPAIR):
                        z2 = zpool.tile([128, T], F32, name=f"z2_{j2}_{p}",
                                        tag="z")
                        h1 = h1s.pop((j2, p))
                        for s in range(2):
                            nc.tensor.matmul(z2[:, s * 512:(s + 1) * 512],
                                             w1_sb[p],
                                             h1[:, s * 512:(s + 1) * 512],
                                             start=True, stop=True)
                        h2 = h2pool.tile([128, T], BF, name=f"h2_{j2}_{p}",
                                         tag="h2")
                        relu(h2, z2, b1_sb[p])
                        h2s[(j2, p)] = h2

    nc.compile()
    return nc


def _prep_inputs(x, mask_head_params, num_ins):
    x = np.asarray(x, dtype=np.float32)
    params = np.asarray(mask_head_params, dtype=np.float32)
    num_ins = np.asarray(num_ins)
    img_idx = np.repeat(np.arange(N_IMG), num_ins)
    assert img_idx.shape[0] == N_IMG * INS_PER_IMG

    xbf = x.reshape(N_IMG, C, L).astype(bf16)

    # coord rows, periodic with T=1024: loc_x = col % 256 and the loc_y
    # in-chunk base (col // 256) % 4; chunk offsets fold into the L1 bias.
    cols = np.arange(LC)
    coords = np.stack([cols % W, (cols // W) % 4]).astype(bf16)

    in_maps = []
    for c in range(N_CORES):
        img, half = c // 2, c % 2
        inst = [img * INS_PER_IMG + k for k in range(INS_PER_IMG)]
        xbc = np.empty((CIN, LC), dtype=bf16)
        xbc[0:C] = xbf[img][:, half * LC:(half + 1) * LC]
        xbc[C:CIN] = coords
        m = {"xb": xbc}
        w0cat = np.zeros((CIN, NPAIR * 128), np.float32)
        wbf = np.zeros((128, NPAIR * (128 + 512)), np.float32)
        wf32 = np.zeros((128, NPAIR * (NCHUNK + 1) + 1), np.float32)
        for p in range(NPAIR):
            a, b = inst[2 * p], inst[2 * p + 1]
            w0_a = params[a, :L1].reshape(C, CIN)
            w0_b = params[b, :L1].reshape(C, CIN)
            # lhsT rows: [w0[:,2:].T ; w0[:,0] (loc_x) ; w0[:,1] (loc_y)]
            for k, wv in enumerate((w0_a, w0_b)):
                colsl = slice(p * 128 + 64 * k, p * 128 + 64 * k + 64)
                w0cat[0:C, colsl] = wv[:, 2:].T
                w0cat[C, colsl] = wv[:, 0]
                w0cat[C + 1, colsl] = wv[:, 1]

            # per-chunk L1 bias: b0 + w0y * (80*half + 4*i)
            b0pair = np.concatenate([params[a, B1OFF:B1OFF + C],
                                     params[b, B1OFF:B1OFF + C]])
            w0y = np.concatenate([w0_a[:, 1], w0_b[:, 1]])
            ii = np.arange(NCHUNK, dtype=np.float32)
            wf32[:, p * NCHUNK:(p + 1) * NCHUNK] = (
                b0pair[:, None] +
                w0y[:, None] * (80.0 * half + 4.0 * ii[None, :]))

            w1_a = params[a, L1:L1 + L2_].reshape(C, C)
            w1_b = params[b, L1:L1 + L2_].reshape(C, C)
            wbf[:64, p * 128:p * 128 + 64] = w1_a.T
            wbf[64:, p * 128 + 64:(p + 1) * 128] = w1_b.T

            w2_a = params[a, L1 + L2_:L1 + L2_ + C]
            w2_b = params[b, L1 + L2_:L1 + L2_ + C]
            w2pair = np.zeros((128, 2), np.float32)
            w2pair[:64, 0] = w2_a
            w2pair[64:, 1] = w2_b
            base = NPAIR * 128 + p * 512
            for j in range(16):
                wbf[:, base + 34 * j:base + 34 * j + 2] = w2pair

            wf32[:, NPAIR * NCHUNK + p] = np.concatenate(
                [params[a, B1OFF + C:B1OFF + 2 * C],
                 params[b, B1OFF + C:B1OFF + 2 * C]])

        # b2 per out-bank partition q = 32a + 16bb + (4p + 2s + m):
        # instance = 2p + m with p = (q%16)//4, m = q%2
        for q in range(128):
            p = (q % 16) // 4
            mm = q % 2
            iid = inst[2 * p + mm]
            wf32[q, NPAIR * (NCHUNK + 1)] = (params[iid, B1OFF + 2 * C]
                                             - MASK_BIAS_SHIFT)
        m["w0c"] = w0cat.astype(bf16)
        m["wbf"] = wbf.astype(bf16)
        m["wf32"] = wf32
        in_maps.append(m)
    return in_maps


def kernel(x, mask_head_params, num_ins):
    if "nc" not in _cache:
        _cache["nc"] = _build_program()
    nc = _cache["nc"]
    in_maps = _prep_inputs(x, mask_head_params, num_ins)
    res = run_bass_kernel_spmd(nc, in_maps, core_ids=list(range(N_CORES)))
    # un-permute packed output: row b*128 + q holds (chunk 8b+2a+bb,
    # pair p, half s, inst-in-pair m) with q = 32a + 16bb + 4p + 2s + m
    q = np.arange(128)
    a, bb, cc = q // 32, (q % 32) // 16, q % 16
    p, s, m = cc // 4, (cc % 4) // 2, cc % 2
    inst_of_q = 2 * p + m
    out = np.empty((N_IMG * INS_PER_IMG, L), dtype=np.float32)
    for c in range(N_CORES):
        img, half = c // 2, c % 2
        pk = res.results[c]["o"].reshape(N_BANKS, 128, 512)
        oc = np.empty((INS_PER_IMG, LC), dtype=np.float32)
        for b in range(N_BANKS):
            chunk = 8 * b + 2 * a + bb
            valid = chunk < NCHUNK
            base = chunk * T + s * 512
            for qi in range(128):
                if valid[qi]:
                    oc[inst_of_q[qi], base[qi]:base[qi] + 512] = pk[b, qi]
        out[img * INS_PER_IMG:(img + 1) * INS_PER_IMG,
            half * LC:(half + 1) * LC] = oc
    return out.reshape(1, N_IMG * INS_PER_IMG, H, W).astype(np.float32)
